# revision 19
# baseline (speedup 1.0000x reference)
"""Multi-head attention (RoPE, causal) Trainium2 kernel.

Problem: B=2, L=2048, D=2048, H=16, dh=128, fp32.
Sharding: 8 cores = 2 batches x 4 head-groups (4 heads/core).
Each core computes QKV projections for its heads, RoPE, causal
attention, and a partial output projection (its heads' rows of Wo);
the host sums the 4 partials per batch.

Layout strategy (no on-device transposes of activations):
 - host uploads xT = x[b].T; Q/K produced transposed [d, l]; V natural
   [l, d]; scores computed transposed ST[k, q]; exp(ST) in [k, q] is
   directly the moving operand of the AV matmul with V as stationary,
   giving UT[d, q] - exactly the Wo-matmul stationary layout.
 - softmax without max subtraction (scores bounded, exp in bf16 whose
   range covers e^60). Row sums via matmuls with an ALL-ONES [128,128]
   stationary, accumulated in PSUM alongside AV: the sum lands
   replicated across all 128 PSUM partitions, so 1/r needs no
   partition broadcast - one DVE reciprocal + one DVE multiply
   normalizes UT during eviction. Full-width k-tiles are summed in
   bf16 pairs then quads on DVE so they need only one row-sum
   matmul per 4 k-tiles; diagonal (causal-masked) tiles keep per-tile
   row sums.
 - causal diagonal blocks are trapezoids: ST/AV/row-sum matmuls
   restrict the moving operand to valid q >= 128*j, and exp+mask cover
   only that range. All 4 diagonal masks are windows of ONE [128,512]
   tril tile (mask[j][k,q]=(k<=q-128j) = mask0[k,q-128j]); the mask
   multiply runs on gpsimd (SBUF-only) to keep DVE off that chain.
 - RoPE rotate-half folded into the sin multiply: with the host-side
   even/odd deinterleave of Wq/Wk rows, rot(x)=[-x_odd; x_even] is a
   +-64-partition shift, done via DVE ops whose OUTPUT partition base
   differs from the (equal-base) input partitions - legal on HW (the
   verifier only requires the two SBUF inputs to share a base). The
   sin table's upper half is negated on device, so per strip RoPE is
   4 DVE ops reading the projection PSUM directly (no scalar
   eviction, no rotate matmul, no permutation constant).
 - cos/sin stream as [64, L] halves; the duplicated cos upper half
   and negated sin upper half are derived on DVE at startup. GpSimd
   is compute-useless here (~18 G elem/s, ~1us op overhead, no PSUM
   access, and its in-order queue delays the DMA issues behind it);
   it only issues wk/wv DMA descriptors.

Dtypes: Q/K side (x, Wq, Wk, Wv, cos/sin, Q, K) in fp16; P/V side
(exp, V, at, Wo) in bf16 (exp needs bf16 range); PSUM f32. Measured
end-to-end error ~2.5e-3 vs the 2e-2 gate. fp16/bf16 matmuls run at
the same PE rate as f32r but halve DMA and SBUF, so ALL weights are
SBUF-resident, loaded once at startup across three DGE queues
(scalar/gpsimd/sync) staggered behind c0's x stream; after chunk 0
the kernel streams only x (2MB/chunk, host pre-tiled to [c][128,
KT*CHUNK] so each chunk is ONE full-bandwidth DMA prefetched during
the previous chunk's attention) and is never DMA-paced. Chunk 0's
ramp is aggregate-HBM-bound (~6.6MB of x+weights before qk can
finish).

Perf structure:
 - attention processes heads in pairs with a one-iteration skew
   between the ST-matmul/exp stage and the AV/rowsum stage, so the
   tensor engine never stalls on the exp latency (stalls reset the PE
   p-state ramp: 1.2GHz for 3us after every gap, 2.4GHz after 3us of
   continuous execution).
 - PSUM: 3 "big" banks (proj accumulators / UT / out-proj) + 3 "st"
   banks (score tiles / out-proj) + 2 "rb" banks (row sums) = 8.
 - out-proj evictions alternate DVE/scalar and banks alternate
   big/st (6-deep rotation); stores alternate sync/scalar DGE queues;
   output partials stored bf16 (host sums in f32). The first three
   out-proj tiles compute their pair-0 half as complete PSUM groups
   in the then-idle "st" banks so the tensor engine has work while
   pair-1's normalization chain lands.

Hardware pitfall found on the way: splitting one PSUM accumulation
group's matmuls into two rounds with other start=True matmuls to
OTHER banks interleaved between them corrupts results on HW (CoreSim
accepts it); keep each tile's accumulation contiguous per bank.
GpSimd (Pool) cannot access PSUM (verifier NCC_IBIR); SBUF-SBUF only.
"""
import sys
import numpy as np

sys.path.insert(0, '/opt/trn_rl_repo')

import concourse.bass as bass  # noqa: E402,F401
import concourse.mybir as mybir  # noqa: E402
import concourse.tile as tile  # noqa: E402
from concourse import bacc  # noqa: E402
from concourse import library_config  # noqa: E402
from concourse.bass_utils import run_bass_kernel_spmd  # noqa: E402

B, L, D = 2, 2048, 2048
H, DH = 16, 128
HG = 4           # heads per core
G = H // HG      # head groups (cores per batch)
NCORES = 8
CHUNK = 512      # l-chunk
NCH = L // CHUNK          # 4 chunks
KT = D // 128             # 16 k-tiles over D
LT = L // 128             # 16 l-tiles
ROPE_BASE = 10000.0

f32 = mybir.dt.float32
f32r = mybir.dt.float32r
f16 = mybir.dt.float16
bf16 = mybir.dt.bfloat16

_built = None
PHASES = []


def _stage_weight_loads(nc, kt, wq_t, wq_d, wk_t, wk_d, wv_t, wv_d,
                        cos_t, cos_d, sin_t, sin_d, masks_t, mask_d,
                        ones_c, ones_c_d):
    """One-time weight/constant loads staggered behind c0's x stream.

    ALL weights ride the scalar DGE queue in exact consumption order
    (wq0, cos, sin, wq1-3, wk0-3); queue order itself paces them so
    nothing steals HBM bandwidth from the stream that gates the next
    strip. x quarters + late constants (wv, masks, ones) ride sync.
    """
    if kt == 3:
        nc.scalar.dma_start(out=wq_t[2][:], in_=wq_d[2])
        nc.scalar.dma_start(out=wq_t[3][:], in_=wq_d[3])
    elif kt == 7:
        nc.scalar.dma_start(out=wk_t[0][:], in_=wk_d[0])
        nc.scalar.dma_start(out=wk_t[1][:], in_=wk_d[1])
        nc.scalar.dma_start(out=wk_t[2][:], in_=wk_d[2])
        nc.scalar.dma_start(out=wk_t[3][:], in_=wk_d[3])
    elif kt == 15:
        # sync queue: behind all four x quarters by construction
        nc.sync.dma_start(out=wv_t[:, :8], in_=wv_d[:8].rearrange("k p f -> p k f"))
        nc.sync.dma_start(out=wv_t[:, 8:], in_=wv_d[8:].rearrange("k p f -> p k f"))
        nc.sync.dma_start(out=masks_t[:], in_=mask_d[:])
        nc.sync.dma_start(out=ones_c[:], in_=ones_c_d[:])


def _build():
    nc = bacc.Bacc()

    # xt: [c][p][kt*CHUNK+n] = x[b].T[kt*128+p, c*CHUNK+n] (host pre-tiled
    # so every DMA line is >=4KB contiguous per partition)
    xt_d = nc.declare_dram_parameter("xt", [NCH, 128, KT * CHUNK], f16,
                                     isOutput=False)
    # wq/wk: [m][p][kt*128+f] = W^T[kt*128+p, m*128+f]
    wq_d = nc.declare_dram_parameter("wq", [HG, 128, KT * 128], f16, isOutput=False)
    wk_d = nc.declare_dram_parameter("wk", [HG, 128, KT * 128], f16, isOutput=False)
    # wv: [kt][p][f] = Wv^T[kt*128+p, f]
    wv_d = nc.declare_dram_parameter("wv", [KT, 128, HG * 128], f16, isOutput=False)
    wo_d = nc.declare_dram_parameter("wo", [HG, 128, D], bf16, isOutput=False)
    cos_d = nc.declare_dram_parameter("cosT", [64, L], f16, isOutput=False)
    sin_d = nc.declare_dram_parameter("sinT", [64, L], f16, isOutput=False)
    mask_d = nc.declare_dram_parameter("masks", [128, CHUNK], bf16, isOutput=False)
    ones_c_d = nc.declare_dram_parameter("ones_c", [128, 128], bf16, isOutput=False)

    out_d = nc.declare_dram_parameter("out", [L, D], bf16, isOutput=True)

    with tile.TileContext(nc) as tc:
        with (
            tc.tile_pool(name="const", bufs=1) as const,
            tc.tile_pool(name="persist", bufs=1) as persist,
            tc.tile_pool(name="xs", bufs=2) as xs,            # flat x tiles
            tc.tile_pool(name="chact", bufs=4) as chact,      # per-chunk qt/at
            tc.tile_pool(name="tmps", bufs=2) as tmps,        # transients
            tc.tile_pool(name="etp", bufs=6) as etp,          # exp tiles (bf16)
            tc.tile_pool(name="small", bufs=2) as small,      # [1,512] tiles
            tc.tile_pool(name="ps", bufs=1, space="PSUM") as pp,
        ):
            # ---- resident weights ----
            wq_t = [persist.tile([128, KT * 128], f16, name=f"wqt{m}")
                    for m in range(HG)]
            wk_t = [persist.tile([128, KT * 128], f16, name=f"wkt{m}")
                    for m in range(HG)]
            wv_t = persist.tile([128, KT, HG * 128], f16, name="wvt")
            wo_t = [persist.tile([128, D], bf16, name=f"wot{h}") for h in range(HG)]
            # ---- constants ----
            cos_t = const.tile([128, L], f16)
            sin_t = const.tile([128, L], f16)
            masks_t = const.tile([128, CHUNK], bf16)
            ones_c = const.tile([128, 128], bf16)

            # scalar queue in consumption order: wq0 gates the first
            # strip; cos/sin gate its rope; then wq1-3 (wk0-3 staggered
            # later). DVE derives the duplicated cos / negated sin upper
            # halves well before the first strip's rope ops need them.
            nc.scalar.dma_start(out=wq_t[0][:], in_=wq_d[0])
            nc.scalar.dma_start(out=wq_t[1][:], in_=wq_d[1])
            nc.scalar.dma_start(out=cos_t[0:64, :], in_=cos_d[:])
            nc.scalar.dma_start(out=sin_t[0:64, :], in_=sin_d[:])
            nc.vector.tensor_copy(out=cos_t[64:128, :], in_=cos_t[0:64, :])
            nc.vector.tensor_scalar(out=sin_t[64:128, :], in0=sin_t[0:64, :],
                                    scalar1=-1.0, scalar2=None,
                                    op0=mybir.AluOpType.mult)

            # ---- persistent activations (full history) ----
            kt_t = [persist.tile([128, L], f16, name=f"ktt{h}") for h in range(HG)]
            v_t = [persist.tile([128, HG * 128], bf16, name=f"vt{lt}")
                   for lt in range(LT)]

            xf_next = None
            for c in range(NCH):
                PHASES.append((f"c{c}_load", int(nc.next_id())))
                cs = slice(c * CHUNK, (c + 1) * CHUNK)
                # ---------- x for chunk c (flat pre-tiled, 4KB lines) ------
                if c == 0:
                    xf = xs.tile([128, KT * CHUNK], f16, tag="xf", name="xf0")
                    for q in range(4):
                        nc.sync.dma_start(
                            out=xf[:, q * 4 * CHUNK:(q + 1) * 4 * CHUNK],
                            in_=xt_d[0, :, q * 4 * CHUNK:(q + 1) * 4 * CHUNK])
                        _stage_weight_loads(nc, 4 * q + 3, wq_t, wq_d, wk_t,
                                            wk_d, wv_t, wv_d, cos_t, cos_d,
                                            sin_t, sin_d, masks_t, mask_d,
                                            ones_c, ones_c_d)
                else:
                    xf = xf_next
                xc = [xf[:, kt * CHUNK:(kt + 1) * CHUNK] for kt in range(KT)]

                PHASES.append((f"c{c}_qk", int(nc.next_id())))
                # ---------- Q/K projections + RoPE ----------
                qt_c = [chact.tile([128, CHUNK], f16, tag="qtc", name=f"qtc{h}")
                        for h in range(HG)]
                for (w_t_, isq) in ((wq_t, True), (wk_t, False)):
                    for m in range(HG):
                        wm = w_t_[m]
                        ps = pp.tile([128, CHUNK], f32, tag="big", bufs=3)
                        for kt in range(KT):
                            nc.tensor.matmul(ps[:], wm[:, kt * 128:(kt + 1) * 128],
                                             xc[kt][:],
                                             start=(kt == 0), stop=(kt == KT - 1))
                        # RoPE fold: out = raw*cos + shift64(raw)*(+-sin).
                        # t2's DVE ops shift the OUTPUT partition base; the
                        # sin table's upper half holds -sin.
                        t1 = tmps.tile([128, CHUNK], f16, tag="t1")
                        nc.vector.tensor_tensor(out=t1[:], in0=ps[:],
                                                in1=cos_t[:, cs],
                                                op=mybir.AluOpType.mult)
                        t2 = tmps.tile([128, CHUNK], f16, tag="t2")
                        nc.vector.tensor_tensor(out=t2[0:64, :],
                                                in0=ps[64:128, :],
                                                in1=sin_t[64:128, cs],
                                                op=mybir.AluOpType.mult)
                        nc.vector.tensor_tensor(out=t2[64:128, :],
                                                in0=ps[0:64, :],
                                                in1=sin_t[0:64, cs],
                                                op=mybir.AluOpType.mult)
                        dst = qt_c[m] if isq else kt_t[m]
                        dst_ap = dst[:] if isq else dst[:, cs]
                        nc.vector.tensor_tensor(out=dst_ap, in0=t1[:], in1=t2[:],
                                                op=mybir.AluOpType.add)

                nkt = (c + 1) * (CHUNK // 128)   # causal: k-tiles 0..nkt-1
                ndiag = nkt - 4        # non-diag kts (multiple of 4)

                def emit_st(kt, hs):
                    """Score matmuls + exp (+ mask) for one kt, both heads
                    of a pair; returns the (et, q0) entry."""
                    # double-wide exp tile: [128, head, q]
                    et = etp.tile([128, 2, CHUNK], bf16, tag="et")
                    diag_j = kt - ndiag
                    q0 = max(diag_j, 0) * 128   # trapezoid: valid q >= q0
                    for hi, h in enumerate(hs):
                        st = pp.tile([128, CHUNK], f32, tag="st", bufs=3)
                        nc.tensor.matmul(
                            st[:, q0:], kt_t[h][:, kt * 128:(kt + 1) * 128],
                            qt_c[h][:, q0:], start=True, stop=True)
                        if diag_j >= 0:
                            # the tril mask is identity for q >= q0+128: exp
                            # writes that wide region directly, and only the
                            # 128-wide slab [q0, q0+128) is masked (first
                            # window of the shared tril tile)
                            if q0 + 128 < CHUNK:
                                nc.scalar.activation(
                                    et[:, hi, q0 + 128:],
                                    st[:, q0 + 128:],
                                    mybir.ActivationFunctionType.Exp)
                            eraw = etp.tile([128, 128], bf16, tag="eraw",
                                            bufs=2)
                            nc.scalar.activation(
                                eraw[:], st[:, q0:q0 + 128],
                                mybir.ActivationFunctionType.Exp)
                            nc.vector.tensor_tensor(
                                out=et[:, hi, q0:q0 + 128],
                                in0=eraw[:], in1=masks_t[:, :128],
                                op=mybir.AluOpType.mult)
                        else:
                            nc.scalar.activation(
                                et[:, hi, q0:], st[:, q0:],
                                mybir.ActivationFunctionType.Exp)
                    return (et, q0)

                PHASES.append((f"c{c}_v", int(nc.next_id())))
                # ---------- V projection ----------
                pre_et = None
                for sl in range(CHUNK // 128):
                    lt = c * (CHUNK // 128) + sl
                    if sl == CHUNK // 128 - 1:
                        # pair-0 kt=0 scores ahead of the last V strip: the
                        # exps land while V finishes, so attention starts
                        # with no warmup bubble
                        pre_et = emit_st(0, (0, 1))
                    ps = pp.tile([128, HG * 128], f32, tag="big", bufs=3)
                    for kt in range(KT):
                        nc.tensor.matmul(
                            ps[:], xc[kt][:, sl * 128:(sl + 1) * 128],
                            wv_t[:, kt, :],
                            start=(kt == 0), stop=(kt == KT - 1))
                    nc.vector.tensor_copy(out=v_t[lt][:], in_=ps[:])

                PHASES.append((f"c{c}_attn", int(nc.next_id())))
                if c + 1 < NCH:
                    xf_next = xs.tile([128, KT * CHUNK], f16, tag="xf",
                                      name=f"xf{c + 1}")
                    nc.sync.dma_start(out=xf_next[:], in_=xt_d[c + 1])
                # ---------- attention for q-chunk c (head pairs, skewed) ----
                at_c = [chact.tile([128, CHUNK], bf16, tag="atc", name=f"atc{h}")
                        for h in range(HG)]
                for pair in range(2):
                    hs = (2 * pair, 2 * pair + 1)
                    ut = {h: pp.tile([128, CHUNK], f32, tag="big", bufs=3,
                                     name=f"ut{h}") for h in hs}
                    rs = {h: pp.tile([128, CHUNK], f32, tag="rb", bufs=2,
                                     name=f"rs{h}") for h in hs}
                    ets = {}
                    quads = {}
                    cums = {}
                    last_eps = None
                    for kt in range(nkt + 1):
                        if kt < nkt:
                            if kt == 0:
                                ets[0] = pre_et
                            else:
                                ets[kt] = emit_st(kt, hs)
                            if pair == 0 and kt == nkt - 1:
                                # pair-1's kt=0 scores ahead of pair-0's
                                # tail, hiding its warmup exp latency
                                pre_et = emit_st(0, (2, 3))
                            # bf16 pair/quad sums of full-width kts; one
                            # row-sum matmul per 4 kts
                            if kt % 2 == 1 and kt < ndiag:
                                eps = etp.tile([128, 2, CHUNK], bf16, tag="eps",
                                               bufs=3)
                                nc.vector.tensor_tensor(
                                    out=eps[:], in0=ets[kt - 1][0][:],
                                    in1=ets[kt][0][:], op=mybir.AluOpType.add)
                                if kt % 4 == 1:
                                    last_eps = eps
                                else:
                                    eqs = etp.tile([128, 2, CHUNK], bf16,
                                                   tag="eqs", bufs=2)
                                    nc.vector.tensor_tensor(
                                        out=eqs[:], in0=last_eps[:], in1=eps[:],
                                        op=mybir.AluOpType.add)
                                    quads[kt] = eqs
                            # cumulative diag sums: region [128i, 128(i+1))
                            # of the row sum needs only diag tiles j <= i, so
                            # each 128-slab gets one narrow rowsum matmul
                            if kt - ndiag >= 1:
                                q0 = (kt - ndiag) * 128
                                prev = (cums[kt - 1] if kt - ndiag >= 2
                                        else ets[ndiag][0])
                                cum = etp.tile([128, 2, CHUNK], bf16,
                                               tag="cum", bufs=2)
                                nc.vector.tensor_tensor(
                                    out=cum[:, :, q0:], in0=prev[:, :, q0:],
                                    in1=ets[kt][0][:, :, q0:],
                                    op=mybir.AluOpType.add)
                                cums[kt] = cum
                        if kt >= 1:
                            e, eq0 = ets.pop(kt - 1)
                            first, last = kt - 1 == 0, kt - 1 == nkt - 1
                            for hi, h in enumerate(hs):
                                nc.tensor.matmul(
                                    ut[h][:, eq0:],
                                    v_t[kt - 1][:, h * 128:(h + 1) * 128],
                                    e[:, hi, eq0:],
                                    start=first, stop=last)
                            if kt - 1 < ndiag:
                                if (kt - 1) % 4 == 3:
                                    # row sums via the quad-sum
                                    eqs = quads.pop(kt - 1)
                                    for hi, h in enumerate(hs):
                                        nc.tensor.matmul(
                                            rs[h][:], ones_c[:],
                                            eqs[:, hi, :],
                                            start=(kt - 1 == 3), stop=False)
                            else:
                                # one 128-wide rowsum per diag slab from the
                                # cumulative sum (d0 itself for slab 0)
                                i = kt - 1 - ndiag
                                src = e if i == 0 else cums.pop(kt - 1)
                                for hi, h in enumerate(hs):
                                    nc.tensor.matmul(
                                        rs[h][:, 128 * i:128 * (i + 1)],
                                        ones_c[:],
                                        src[:, hi, 128 * i:128 * (i + 1)],
                                        start=(first or
                                               (ndiag == 0 and i == 0)),
                                        stop=last)
                    for hi, h in enumerate(hs):
                        rb_sb = tmps.tile([128, CHUNK], f32, tag="bc", bufs=2)
                        nc.vector.reciprocal_approx_fast(out=rb_sb[:],
                                                         in_=rs[h][:])
                        nc.vector.tensor_tensor(out=at_c[h][:], in0=ut[h][:],
                                                in1=rb_sb[:],
                                                op=mybir.AluOpType.mult)

                PHASES.append((f"c{c}_out", int(nc.next_id())))
                # ---------- output projection for chunk c ----------
                if c == 0:
                    for h in range(HG):
                        nc.sync.dma_start(out=wo_t[h][:], in_=wo_d[h])
                # first five tiles: pair-0 halves as COMPLETE groups run
                # before pair-1's normalization chain (2 recip + 2 mult on
                # DVE, ~2.4us) lands; halves summed on DVE at eviction.
                # "st" banks are free during out-proj, so these half-groups
                # need no "big" bank (those are freed BY the chain).
                NHALF = 5
                half_a = {}
                for idx in range(NHALF):
                    ot, sl = divmod(idx, 4)
                    opa = pp.tile([128, 512], f32, tag="st", bufs=3,
                                  name=f"opa{idx}")
                    for h in (0, 1):
                        nc.tensor.matmul(
                            opa[:], at_c[h][:, sl * 128:(sl + 1) * 128],
                            wo_t[h][:, ot * 512:(ot + 1) * 512],
                            start=(h == 0), stop=(h == 1))
                    osa = tmps.tile([128, 512], f32, tag="osba", bufs=NHALF)
                    nc.scalar.copy(out=osa[:], in_=opa[:])
                    half_a[idx] = osa
                for idx in range(16):
                    ot, sl = divmod(idx, 4)
                    mt = c * (CHUNK // 128) + sl
                    osb = tmps.tile([128, 512], bf16, tag="osb", bufs=6)
                    if idx < NHALF:
                        opb = pp.tile([128, 512], f32, tag="big", bufs=3)
                        for h in (2, 3):
                            nc.tensor.matmul(
                                opb[:], at_c[h][:, sl * 128:(sl + 1) * 128],
                                wo_t[h][:, ot * 512:(ot + 1) * 512],
                                start=(h == 2), stop=(h == 3))
                        nc.vector.tensor_tensor(out=osb[:], in0=half_a[idx][:],
                                                in1=opb[:],
                                                op=mybir.AluOpType.add)
                    else:
                        # banks alternate big/st (6-deep rotation);
                        # evictions alternate DVE/scalar
                        ops = pp.tile([128, 512], f32,
                                      tag=("big" if idx % 2 else "st"), bufs=3)
                        for h in range(HG):
                            nc.tensor.matmul(
                                ops[:], at_c[h][:, sl * 128:(sl + 1) * 128],
                                wo_t[h][:, ot * 512:(ot + 1) * 512],
                                start=(h == 0), stop=(h == HG - 1))
                        if idx % 2:
                            nc.vector.tensor_copy(out=osb[:], in_=ops[:])
                        else:
                            nc.scalar.copy(out=osb[:], in_=ops[:])
                    qeng = nc.scalar if idx % 4 == 1 else nc.sync
                    qeng.dma_start(
                        out=out_d[mt * 128:(mt + 1) * 128, ot * 512:(ot + 1) * 512],
                        in_=osb[:])

    nc.finalize()
    return nc


def _get_nc():
    global _built
    if _built is None:
        _built = _build()
    return _built


def _host_prep(x, positions, Wq, Wk, Wv, Wo):
    """Build per-core input maps."""
    import ml_dtypes
    x = np.asarray(x, np.float32)
    positions = np.asarray(positions)
    Wq = np.asarray(Wq, np.float32)
    Wk = np.asarray(Wk, np.float32)
    Wv = np.asarray(Wv, np.float32)
    Wo = np.asarray(Wo, np.float32)

    scale = np.float32(1.0 / np.sqrt(DH))
    perm = np.concatenate([np.arange(0, DH, 2), np.arange(1, DH, 2)])  # deinterleave

    Wq_p = (Wq * scale).reshape(H, DH, D)[:, perm, :]   # [H, dh, D]
    Wk_p = Wk.reshape(H, DH, D)[:, perm, :]

    # RoPE tables per batch, [64, L] halves (device derives dup/neg)
    inv_freq = 1.0 / (ROPE_BASE ** (np.arange(0, DH, 2, dtype=np.float32) / DH))
    cosT = np.empty((B, 64, L), np.float32)
    sinT = np.empty((B, 64, L), np.float32)
    for b in range(B):
        freqs = positions[b].astype(np.float32)[:, None] * inv_freq[None, :]  # [L, 64]
        cosT[b] = np.cos(freqs).T.astype(np.float32)  # [64, L]
        sinT[b] = np.sin(freqs).T.astype(np.float32)

    # single tril tile; diagonal block j's mask is its width-(512-128j) window
    kk = np.arange(128)[:, None]
    qq = np.arange(CHUNK)[None, :]
    mask0 = (kk <= qq).astype(np.float32).astype(ml_dtypes.bfloat16)

    ones_c = np.ones((128, 128), ml_dtypes.bfloat16)

    in_maps = []
    for core in range(NCORES):
        b, g = divmod(core, G)
        hs = slice(g * HG, (g + 1) * HG)
        # W^T for this core's heads: [D, HG*dh]
        wqT = Wq_p[hs].reshape(HG * DH, D).T          # [D, 512]
        wkT = Wk_p[hs].reshape(HG * DH, D).T
        wvT = Wv.reshape(H, DH, D)[hs].reshape(HG * DH, D).T
        # [m][p][kt*128+f] layout
        wq_c = np.ascontiguousarray(
            wqT.reshape(KT, 128, HG, DH).transpose(2, 1, 0, 3).reshape(
                HG, 128, KT * DH)).astype(np.float16)
        wk_c = np.ascontiguousarray(
            wkT.reshape(KT, 128, HG, DH).transpose(2, 1, 0, 3).reshape(
                HG, 128, KT * DH)).astype(np.float16)
        # [kt][p][f]
        wv_c = np.ascontiguousarray(
            wvT.reshape(KT, 128, HG * DH)).astype(np.float16)
        # wo[h][d'][o] = Wo[o, (g*HG+h)*dh + d']
        wo_c = np.ascontiguousarray(
            Wo.T.reshape(H, DH, D)[hs]).astype(ml_dtypes.bfloat16)  # [HG, dh, D]
        xtb = x[b].T.astype(np.float16)   # [D, L]
        xt_tiled = np.ascontiguousarray(
            xtb.reshape(KT, 128, NCH, CHUNK).transpose(2, 1, 0, 3).reshape(
                NCH, 128, KT * CHUNK))
        in_maps.append({
            "xt": xt_tiled,
            "wq": wq_c, "wk": wk_c, "wv": wv_c, "wo": wo_c,
            "cosT": cosT[b].astype(np.float16),
            "sinT": sinT[b].astype(np.float16),
            "masks": mask0,
            "ones_c": ones_c,
        })
    return in_maps


def kernel(x, positions, Wq, Wk, Wv, Wo, _profile=False):
    nc = _get_nc()
    in_maps = _host_prep(x, positions, Wq, Wk, Wv, Wo)
    res = run_bass_kernel_spmd(nc, in_maps, list(range(NCORES)), trace=_profile)
    out = np.zeros((B, L, D), np.float32)
    for core in range(NCORES):
        b = core // G
        out[b] += np.asarray(res.results[core]["out"], np.float32)
    if _profile:
        kernel._last_exec_time_ns = res.exec_time_ns
        kernel._last_trace = res.instructions_and_trace
    return out


# revision 21
# speedup vs baseline: 1.0058x; 1.0058x over previous
"""Multi-head attention (RoPE, causal) Trainium2 kernel.

Problem: B=2, L=2048, D=2048, H=16, dh=128, fp32.
Sharding: 8 cores = 2 batches x 4 head-groups (4 heads/core).
Each core computes QKV projections for its heads, RoPE, causal
attention, and a partial output projection (its heads' rows of Wo);
the host sums the 4 partials per batch.

Layout strategy (no on-device transposes of activations):
 - host uploads xT = x[b].T; Q/K produced transposed [d, l]; V natural
   [l, d]; scores computed transposed ST[k, q]; exp(ST) in [k, q] is
   directly the moving operand of the AV matmul with V as stationary,
   giving UT[d, q] - exactly the Wo-matmul stationary layout.
 - softmax without max subtraction (scores bounded, exp in bf16 whose
   range covers e^60). Row sums via matmuls with an ALL-ONES [128,128]
   stationary, accumulated in PSUM alongside AV: the sum lands
   replicated across all 128 PSUM partitions, so 1/r needs no
   partition broadcast - one DVE reciprocal + one DVE multiply
   normalizes UT during eviction. Full-width k-tiles are summed in
   bf16 pairs then quads on DVE so they need only one row-sum
   matmul per 4 k-tiles; diagonal (causal-masked) tiles keep per-tile
   row sums.
 - causal diagonal blocks are trapezoids: ST/AV/row-sum matmuls
   restrict the moving operand to valid q >= 128*j, and exp+mask cover
   only that range. All 4 diagonal masks are windows of ONE [128,512]
   tril tile (mask[j][k,q]=(k<=q-128j) = mask0[k,q-128j]); the mask
   multiply runs on gpsimd (SBUF-only) to keep DVE off that chain.
 - RoPE rotate-half folded into the sin multiply: with the host-side
   even/odd deinterleave of Wq/Wk rows, rot(x)=[-x_odd; x_even] is a
   +-64-partition shift, done via DVE ops whose OUTPUT partition base
   differs from the (equal-base) input partitions - legal on HW (the
   verifier only requires the two SBUF inputs to share a base). The
   sin table's upper half is negated on device, so per strip RoPE is
   4 DVE ops reading the projection PSUM directly (no scalar
   eviction, no rotate matmul, no permutation constant).
 - cos/sin stream as [64, L] halves; the duplicated cos upper half
   and negated sin upper half are derived on DVE at startup. GpSimd
   is compute-useless here (~18 G elem/s, ~1us op overhead, no PSUM
   access, and its in-order queue delays the DMA issues behind it);
   it only issues wk/wv DMA descriptors.

Dtypes: Q/K side (x, Wq, Wk, Wv, cos/sin, Q, K) in fp16; P/V side
(exp, V, at, Wo) in bf16 (exp needs bf16 range); PSUM f32. Measured
end-to-end error ~2.5e-3 vs the 2e-2 gate. fp16/bf16 matmuls run at
the same PE rate as f32r but halve DMA and SBUF, so ALL weights are
SBUF-resident, loaded once at startup across three DGE queues
(scalar/gpsimd/sync) staggered behind c0's x stream; after chunk 0
the kernel streams only x (2MB/chunk, host pre-tiled to [c][128,
KT*CHUNK] so each chunk is ONE full-bandwidth DMA prefetched during
the previous chunk's attention) and is never DMA-paced. Chunk 0's
ramp is aggregate-HBM-bound (~6.6MB of x+weights before qk can
finish).

Perf structure:
 - attention processes heads in pairs with a one-iteration skew
   between the ST-matmul/exp stage and the AV/rowsum stage, so the
   tensor engine never stalls on the exp latency (stalls reset the PE
   p-state ramp: 1.2GHz for 3us after every gap, 2.4GHz after 3us of
   continuous execution).
 - PSUM: 3 "big" banks (proj accumulators / UT / out-proj) + 3 "st"
   banks (score tiles / out-proj) + 2 "rb" banks (row sums) = 8.
 - out-proj evictions alternate DVE/scalar and banks alternate
   big/st (6-deep rotation); stores alternate sync/scalar DGE queues;
   output partials stored bf16 (host sums in f32). The first three
   out-proj tiles compute their pair-0 half as complete PSUM groups
   in the then-idle "st" banks so the tensor engine has work while
   pair-1's normalization chain lands.

Hardware pitfall found on the way: splitting one PSUM accumulation
group's matmuls into two rounds with other start=True matmuls to
OTHER banks interleaved between them corrupts results on HW (CoreSim
accepts it); keep each tile's accumulation contiguous per bank.
GpSimd (Pool) cannot access PSUM (verifier NCC_IBIR); SBUF-SBUF only.
"""
import sys
import numpy as np

sys.path.insert(0, '/opt/trn_rl_repo')

import concourse.bass as bass  # noqa: E402,F401
import concourse.mybir as mybir  # noqa: E402
import concourse.tile as tile  # noqa: E402
from concourse import bacc  # noqa: E402
from concourse import library_config  # noqa: E402
from concourse.bass_utils import run_bass_kernel_spmd  # noqa: E402

B, L, D = 2, 2048, 2048
H, DH = 16, 128
HG = 4           # heads per core
G = H // HG      # head groups (cores per batch)
NCORES = 8
CHUNK = 512      # l-chunk
NCH = L // CHUNK          # 4 chunks
KT = D // 128             # 16 k-tiles over D
LT = L // 128             # 16 l-tiles
ROPE_BASE = 10000.0

f32 = mybir.dt.float32
f32r = mybir.dt.float32r
f16 = mybir.dt.float16
bf16 = mybir.dt.bfloat16

_built = None
PHASES = []


def _stage_weight_loads(nc, kt, wq_t, wq_d, wk_t, wk_d, wv_t, wv_d,
                        cos_t, cos_d, sin_t, sin_d, masks_t, mask_d,
                        ones_c, ones_c_d):
    """One-time weight/constant loads staggered behind c0's x stream.

    ALL weights ride the scalar DGE queue in exact consumption order
    (wq0, cos, sin, wq1-3, wk0-3); queue order itself paces them so
    nothing steals HBM bandwidth from the stream that gates the next
    strip. x quarters + late constants (wv, masks, ones) ride sync.
    """
    if kt == 3:
        nc.scalar.dma_start(out=wq_t[2][:], in_=wq_d[2])
        nc.scalar.dma_start(out=wq_t[3][:], in_=wq_d[3])
    elif kt == 7:
        nc.scalar.dma_start(out=wk_t[0][:], in_=wk_d[0])
        nc.scalar.dma_start(out=wk_t[1][:], in_=wk_d[1])
        nc.scalar.dma_start(out=wk_t[2][:], in_=wk_d[2])
        nc.scalar.dma_start(out=wk_t[3][:], in_=wk_d[3])
    elif kt == 15:
        # sync queue: behind all four x quarters by construction
        nc.sync.dma_start(out=wv_t[:, :8], in_=wv_d[:8].rearrange("k p f -> p k f"))
        nc.sync.dma_start(out=wv_t[:, 8:], in_=wv_d[8:].rearrange("k p f -> p k f"))
        nc.sync.dma_start(out=masks_t[:], in_=mask_d[:])
        nc.sync.dma_start(out=ones_c[:], in_=ones_c_d[:])


def _build():
    nc = bacc.Bacc()

    # xt: [c][p][kt*CHUNK+n] = x[b].T[kt*128+p, c*CHUNK+n] (host pre-tiled
    # so every DMA line is >=4KB contiguous per partition)
    xt_d = nc.declare_dram_parameter("xt", [NCH, 128, KT * CHUNK], f16,
                                     isOutput=False)
    # wq/wk: [m][p][kt*128+f] = W^T[kt*128+p, m*128+f]
    wq_d = nc.declare_dram_parameter("wq", [HG, 128, KT * 128], f16, isOutput=False)
    wk_d = nc.declare_dram_parameter("wk", [HG, 128, KT * 128], f16, isOutput=False)
    # wv: [kt][p][f] = Wv^T[kt*128+p, f]
    wv_d = nc.declare_dram_parameter("wv", [KT, 128, HG * 128], f16, isOutput=False)
    wo_d = nc.declare_dram_parameter("wo", [HG, 128, D], bf16, isOutput=False)
    cos_d = nc.declare_dram_parameter("cosT", [64, L], f16, isOutput=False)
    sin_d = nc.declare_dram_parameter("sinT", [64, L], f16, isOutput=False)
    mask_d = nc.declare_dram_parameter("masks", [128, CHUNK], bf16, isOutput=False)
    ones_c_d = nc.declare_dram_parameter("ones_c", [128, 128], bf16, isOutput=False)

    out_d = nc.declare_dram_parameter("out", [L, D], bf16, isOutput=True)

    with tile.TileContext(nc) as tc:
        with (
            tc.tile_pool(name="const", bufs=1) as const,
            tc.tile_pool(name="persist", bufs=1) as persist,
            tc.tile_pool(name="xs", bufs=2) as xs,            # flat x tiles
            tc.tile_pool(name="chact", bufs=4) as chact,      # per-chunk qt/at
            tc.tile_pool(name="tmps", bufs=2) as tmps,        # transients
            tc.tile_pool(name="etp", bufs=6) as etp,          # exp tiles (bf16)
            tc.tile_pool(name="small", bufs=2) as small,      # [1,512] tiles
            tc.tile_pool(name="ps", bufs=1, space="PSUM") as pp,
        ):
            # ---- resident weights ----
            wq_t = [persist.tile([128, KT * 128], f16, name=f"wqt{m}")
                    for m in range(HG)]
            wk_t = [persist.tile([128, KT * 128], f16, name=f"wkt{m}")
                    for m in range(HG)]
            wv_t = persist.tile([128, KT, HG * 128], f16, name="wvt")
            wo_t = [persist.tile([128, D], bf16, name=f"wot{h}") for h in range(HG)]
            # ---- constants ----
            cos_t = const.tile([128, L], f16)
            sin_t = const.tile([128, L], f16)
            masks_t = const.tile([128, CHUNK], bf16)
            ones_c = const.tile([128, 128], bf16)

            # scalar queue in consumption order: wq0 gates the first
            # strip; cos/sin gate its rope; then wq1-3 (wk0-3 staggered
            # later). DVE derives the duplicated cos / negated sin upper
            # halves well before the first strip's rope ops need them.
            nc.scalar.dma_start(out=wq_t[0][:], in_=wq_d[0])
            nc.scalar.dma_start(out=wq_t[1][:], in_=wq_d[1])
            nc.scalar.dma_start(out=cos_t[0:64, :], in_=cos_d[:])
            nc.scalar.dma_start(out=sin_t[0:64, :], in_=sin_d[:])
            nc.vector.tensor_copy(out=cos_t[64:128, :], in_=cos_t[0:64, :])
            nc.vector.tensor_scalar(out=sin_t[64:128, :], in0=sin_t[0:64, :],
                                    scalar1=-1.0, scalar2=None,
                                    op0=mybir.AluOpType.mult)

            # ---- persistent activations (full history) ----
            kt_t = [persist.tile([128, L], f16, name=f"ktt{h}") for h in range(HG)]
            v_t = [persist.tile([128, HG * 128], bf16, name=f"vt{lt}")
                   for lt in range(LT)]

            xf_next = None
            for c in range(NCH):
                PHASES.append((f"c{c}_load", int(nc.next_id())))
                cs = slice(c * CHUNK, (c + 1) * CHUNK)
                # ---------- x for chunk c (flat pre-tiled, 4KB lines) ------
                if c == 0:
                    xf = xs.tile([128, KT * CHUNK], f16, tag="xf", name="xf0")
                    for q in range(4):
                        nc.sync.dma_start(
                            out=xf[:, q * 4 * CHUNK:(q + 1) * 4 * CHUNK],
                            in_=xt_d[0, :, q * 4 * CHUNK:(q + 1) * 4 * CHUNK])
                        _stage_weight_loads(nc, 4 * q + 3, wq_t, wq_d, wk_t,
                                            wk_d, wv_t, wv_d, cos_t, cos_d,
                                            sin_t, sin_d, masks_t, mask_d,
                                            ones_c, ones_c_d)
                else:
                    xf = xf_next
                xc = [xf[:, kt * CHUNK:(kt + 1) * CHUNK] for kt in range(KT)]

                PHASES.append((f"c{c}_qk", int(nc.next_id())))
                # ---------- Q/K projections + RoPE ----------
                qt_c = [chact.tile([128, CHUNK], f16, tag="qtc", name=f"qtc{h}")
                        for h in range(HG)]
                for (w_t_, isq) in ((wq_t, True), (wk_t, False)):
                    for m in range(HG):
                        wm = w_t_[m]
                        ps = pp.tile([128, CHUNK], f32, tag="big", bufs=3)
                        for kt in range(KT):
                            nc.tensor.matmul(ps[:], wm[:, kt * 128:(kt + 1) * 128],
                                             xc[kt][:],
                                             start=(kt == 0), stop=(kt == KT - 1))
                        # RoPE fold: out = raw*cos + shift64(raw)*(+-sin).
                        # t2's DVE ops shift the OUTPUT partition base; the
                        # sin table's upper half holds -sin.
                        t1 = tmps.tile([128, CHUNK], f16, tag="t1")
                        nc.vector.tensor_tensor(out=t1[:], in0=ps[:],
                                                in1=cos_t[:, cs],
                                                op=mybir.AluOpType.mult)
                        t2 = tmps.tile([128, CHUNK], f16, tag="t2")
                        nc.vector.tensor_tensor(out=t2[0:64, :],
                                                in0=ps[64:128, :],
                                                in1=sin_t[64:128, cs],
                                                op=mybir.AluOpType.mult)
                        nc.vector.tensor_tensor(out=t2[64:128, :],
                                                in0=ps[0:64, :],
                                                in1=sin_t[0:64, cs],
                                                op=mybir.AluOpType.mult)
                        dst = qt_c[m] if isq else kt_t[m]
                        dst_ap = dst[:] if isq else dst[:, cs]
                        nc.vector.tensor_tensor(out=dst_ap, in0=t1[:], in1=t2[:],
                                                op=mybir.AluOpType.add)

                nkt = (c + 1) * (CHUNK // 128)   # causal: k-tiles 0..nkt-1
                ndiag = nkt - 4        # non-diag kts (multiple of 4)

                def emit_st(kt, hs):
                    """Score matmuls + exp (+ mask) for one kt, both heads
                    of a pair; returns the (et, q0) entry."""
                    # double-wide exp tile: [128, head, q]
                    et = etp.tile([128, 2, CHUNK], bf16, tag="et")
                    diag_j = kt - ndiag
                    q0 = max(diag_j, 0) * 128   # trapezoid: valid q >= q0
                    for hi, h in enumerate(hs):
                        st = pp.tile([128, CHUNK], f32, tag="st", bufs=3)
                        nc.tensor.matmul(
                            st[:, q0:], kt_t[h][:, kt * 128:(kt + 1) * 128],
                            qt_c[h][:, q0:], start=True, stop=True)
                        if diag_j >= 0:
                            eraw = etp.tile([128, CHUNK], bf16, tag="eraw",
                                            bufs=2)
                            nc.scalar.activation(
                                eraw[:, q0:], st[:, q0:],
                                mybir.ActivationFunctionType.Exp)
                            # tril window == mask for diag block j; one
                            # writer per et half (two-writer split races
                            # on HW - CoreSim does not catch it)
                            nc.vector.tensor_tensor(
                                out=et[:, hi, q0:], in0=eraw[:, q0:],
                                in1=masks_t[:, :CHUNK - q0],
                                op=mybir.AluOpType.mult)
                        else:
                            nc.scalar.activation(
                                et[:, hi, q0:], st[:, q0:],
                                mybir.ActivationFunctionType.Exp)
                    return (et, q0)

                PHASES.append((f"c{c}_v", int(nc.next_id())))
                # ---------- V projection ----------
                pre_et = None
                for sl in range(CHUNK // 128):
                    lt = c * (CHUNK // 128) + sl
                    if sl == CHUNK // 128 - 1:
                        # pair-0 kt=0 scores ahead of the last V strip: the
                        # exps land while V finishes, so attention starts
                        # with no warmup bubble
                        pre_et = emit_st(0, (0, 1))
                    ps = pp.tile([128, HG * 128], f32, tag="big", bufs=3)
                    for kt in range(KT):
                        nc.tensor.matmul(
                            ps[:], xc[kt][:, sl * 128:(sl + 1) * 128],
                            wv_t[:, kt, :],
                            start=(kt == 0), stop=(kt == KT - 1))
                    nc.vector.tensor_copy(out=v_t[lt][:], in_=ps[:])

                PHASES.append((f"c{c}_attn", int(nc.next_id())))
                if c + 1 < NCH:
                    xf_next = xs.tile([128, KT * CHUNK], f16, tag="xf",
                                      name=f"xf{c + 1}")
                    nc.sync.dma_start(out=xf_next[:], in_=xt_d[c + 1])
                # ---------- attention for q-chunk c (head pairs, skewed) ----
                at_c = [chact.tile([128, CHUNK], bf16, tag="atc", name=f"atc{h}")
                        for h in range(HG)]
                for pair in range(2):
                    hs = (2 * pair, 2 * pair + 1)
                    ut = {h: pp.tile([128, CHUNK], f32, tag="big", bufs=3,
                                     name=f"ut{h}") for h in hs}
                    rs = {h: pp.tile([128, CHUNK], f32, tag="rb", bufs=2,
                                     name=f"rs{h}") for h in hs}
                    ets = {}
                    quads = {}
                    cums = {}
                    last_eps = None
                    for kt in range(nkt + 1):
                        if kt < nkt:
                            if kt == 0:
                                ets[0] = pre_et
                            else:
                                ets[kt] = emit_st(kt, hs)
                            if pair == 0 and kt == nkt - 1:
                                # pair-1's kt=0 scores ahead of pair-0's
                                # tail, hiding its warmup exp latency
                                pre_et = emit_st(0, (2, 3))
                            # bf16 pair/quad sums of full-width kts; one
                            # row-sum matmul per 4 kts
                            if kt % 2 == 1 and kt < ndiag:
                                eps = etp.tile([128, 2, CHUNK], bf16, tag="eps",
                                               bufs=3)
                                nc.vector.tensor_tensor(
                                    out=eps[:], in0=ets[kt - 1][0][:],
                                    in1=ets[kt][0][:], op=mybir.AluOpType.add)
                                if kt % 4 == 1:
                                    last_eps = eps
                                else:
                                    eqs = etp.tile([128, 2, CHUNK], bf16,
                                                   tag="eqs", bufs=2)
                                    nc.vector.tensor_tensor(
                                        out=eqs[:], in0=last_eps[:], in1=eps[:],
                                        op=mybir.AluOpType.add)
                                    quads[kt] = eqs
                        if kt >= 1:
                            e, eq0 = ets.pop(kt - 1)
                            first, last = kt - 1 == 0, kt - 1 == nkt - 1
                            for hi, h in enumerate(hs):
                                nc.tensor.matmul(
                                    ut[h][:, eq0:],
                                    v_t[kt - 1][:, h * 128:(h + 1) * 128],
                                    e[:, hi, eq0:],
                                    start=first, stop=last)
                            if kt - 1 < ndiag:
                                if (kt - 1) % 4 == 3:
                                    # row sums via the quad-sum
                                    eqs = quads.pop(kt - 1)
                                    for hi, h in enumerate(hs):
                                        nc.tensor.matmul(
                                            rs[h][:], ones_c[:],
                                            eqs[:, hi, :],
                                            start=(kt - 1 == 3), stop=False)
                            else:
                                for hi, h in enumerate(hs):
                                    nc.tensor.matmul(
                                        rs[h][:, eq0:], ones_c[:],
                                        e[:, hi, eq0:],
                                        start=first, stop=last)
                    for hi, h in enumerate(hs):
                        rb_sb = tmps.tile([128, CHUNK], f32, tag="bc", bufs=2)
                        nc.vector.reciprocal_approx_fast(out=rb_sb[:],
                                                         in_=rs[h][:])
                        nc.vector.tensor_tensor(out=at_c[h][:], in0=ut[h][:],
                                                in1=rb_sb[:],
                                                op=mybir.AluOpType.mult)

                PHASES.append((f"c{c}_out", int(nc.next_id())))
                # ---------- output projection for chunk c ----------
                if c == 0:
                    for h in range(HG):
                        nc.sync.dma_start(out=wo_t[h][:], in_=wo_d[h])
                # first five tiles: pair-0 halves as COMPLETE groups run
                # before pair-1's normalization chain (2 recip + 2 mult on
                # DVE, ~2.4us) lands; halves summed on DVE at eviction.
                # "st" banks are free during out-proj, so these half-groups
                # need no "big" bank (those are freed BY the chain).
                NHALF = 5
                half_a = {}
                for idx in range(NHALF):
                    ot, sl = divmod(idx, 4)
                    opa = pp.tile([128, 512], f32, tag="st", bufs=3,
                                  name=f"opa{idx}")
                    for h in (0, 1):
                        nc.tensor.matmul(
                            opa[:], at_c[h][:, sl * 128:(sl + 1) * 128],
                            wo_t[h][:, ot * 512:(ot + 1) * 512],
                            start=(h == 0), stop=(h == 1))
                    osa = tmps.tile([128, 512], f32, tag="osba", bufs=NHALF)
                    nc.scalar.copy(out=osa[:], in_=opa[:])
                    half_a[idx] = osa
                for idx in range(16):
                    ot, sl = divmod(idx, 4)
                    mt = c * (CHUNK // 128) + sl
                    osb = tmps.tile([128, 512], bf16, tag="osb", bufs=6)
                    if idx < NHALF:
                        opb = pp.tile([128, 512], f32, tag="big", bufs=3)
                        for h in (2, 3):
                            nc.tensor.matmul(
                                opb[:], at_c[h][:, sl * 128:(sl + 1) * 128],
                                wo_t[h][:, ot * 512:(ot + 1) * 512],
                                start=(h == 2), stop=(h == 3))
                        nc.vector.tensor_tensor(out=osb[:], in0=half_a[idx][:],
                                                in1=opb[:],
                                                op=mybir.AluOpType.add)
                    else:
                        # banks alternate big/st (6-deep rotation);
                        # evictions alternate DVE/scalar
                        ops = pp.tile([128, 512], f32,
                                      tag=("big" if idx % 2 else "st"), bufs=3)
                        for h in range(HG):
                            nc.tensor.matmul(
                                ops[:], at_c[h][:, sl * 128:(sl + 1) * 128],
                                wo_t[h][:, ot * 512:(ot + 1) * 512],
                                start=(h == 0), stop=(h == HG - 1))
                        if idx % 2:
                            nc.vector.tensor_copy(out=osb[:], in_=ops[:])
                        else:
                            nc.scalar.copy(out=osb[:], in_=ops[:])
                    qeng = nc.scalar if idx % 4 == 1 else nc.sync
                    qeng.dma_start(
                        out=out_d[mt * 128:(mt + 1) * 128, ot * 512:(ot + 1) * 512],
                        in_=osb[:])

    nc.finalize()
    return nc


def _get_nc():
    global _built
    if _built is None:
        _built = _build()
    return _built


def _host_prep(x, positions, Wq, Wk, Wv, Wo):
    """Build per-core input maps."""
    import ml_dtypes
    x = np.asarray(x, np.float32)
    positions = np.asarray(positions)
    Wq = np.asarray(Wq, np.float32)
    Wk = np.asarray(Wk, np.float32)
    Wv = np.asarray(Wv, np.float32)
    Wo = np.asarray(Wo, np.float32)

    scale = np.float32(1.0 / np.sqrt(DH))
    perm = np.concatenate([np.arange(0, DH, 2), np.arange(1, DH, 2)])  # deinterleave

    Wq_p = (Wq * scale).reshape(H, DH, D)[:, perm, :]   # [H, dh, D]
    Wk_p = Wk.reshape(H, DH, D)[:, perm, :]

    # RoPE tables per batch, [64, L] halves (device derives dup/neg)
    inv_freq = 1.0 / (ROPE_BASE ** (np.arange(0, DH, 2, dtype=np.float32) / DH))
    cosT = np.empty((B, 64, L), np.float32)
    sinT = np.empty((B, 64, L), np.float32)
    for b in range(B):
        freqs = positions[b].astype(np.float32)[:, None] * inv_freq[None, :]  # [L, 64]
        cosT[b] = np.cos(freqs).T.astype(np.float32)  # [64, L]
        sinT[b] = np.sin(freqs).T.astype(np.float32)

    # single tril tile; diagonal block j's mask is its width-(512-128j) window
    kk = np.arange(128)[:, None]
    qq = np.arange(CHUNK)[None, :]
    mask0 = (kk <= qq).astype(np.float32).astype(ml_dtypes.bfloat16)

    ones_c = np.ones((128, 128), ml_dtypes.bfloat16)

    in_maps = []
    for core in range(NCORES):
        b, g = divmod(core, G)
        hs = slice(g * HG, (g + 1) * HG)
        # W^T for this core's heads: [D, HG*dh]
        wqT = Wq_p[hs].reshape(HG * DH, D).T          # [D, 512]
        wkT = Wk_p[hs].reshape(HG * DH, D).T
        wvT = Wv.reshape(H, DH, D)[hs].reshape(HG * DH, D).T
        # [m][p][kt*128+f] layout
        wq_c = np.ascontiguousarray(
            wqT.reshape(KT, 128, HG, DH).transpose(2, 1, 0, 3).reshape(
                HG, 128, KT * DH)).astype(np.float16)
        wk_c = np.ascontiguousarray(
            wkT.reshape(KT, 128, HG, DH).transpose(2, 1, 0, 3).reshape(
                HG, 128, KT * DH)).astype(np.float16)
        # [kt][p][f]
        wv_c = np.ascontiguousarray(
            wvT.reshape(KT, 128, HG * DH)).astype(np.float16)
        # wo[h][d'][o] = Wo[o, (g*HG+h)*dh + d']
        wo_c = np.ascontiguousarray(
            Wo.T.reshape(H, DH, D)[hs]).astype(ml_dtypes.bfloat16)  # [HG, dh, D]
        xtb = x[b].T.astype(np.float16)   # [D, L]
        xt_tiled = np.ascontiguousarray(
            xtb.reshape(KT, 128, NCH, CHUNK).transpose(2, 1, 0, 3).reshape(
                NCH, 128, KT * CHUNK))
        in_maps.append({
            "xt": xt_tiled,
            "wq": wq_c, "wk": wk_c, "wv": wv_c, "wo": wo_c,
            "cosT": cosT[b].astype(np.float16),
            "sinT": sinT[b].astype(np.float16),
            "masks": mask0,
            "ones_c": ones_c,
        })
    return in_maps


def kernel(x, positions, Wq, Wk, Wv, Wo, _profile=False):
    nc = _get_nc()
    in_maps = _host_prep(x, positions, Wq, Wk, Wv, Wo)
    res = run_bass_kernel_spmd(nc, in_maps, list(range(NCORES)), trace=_profile)
    out = np.zeros((B, L, D), np.float32)
    for core in range(NCORES):
        b = core // G
        out[b] += np.asarray(res.results[core]["out"], np.float32)
    if _profile:
        kernel._last_exec_time_ns = res.exec_time_ns
        kernel._last_trace = res.instructions_and_trace
    return out


# revision 22
# speedup vs baseline: 1.0152x; 1.0094x over previous
"""Multi-head attention (RoPE, causal) Trainium2 kernel.

Problem: B=2, L=2048, D=2048, H=16, dh=128, fp32.
Sharding: 8 cores = 2 batches x 4 head-groups (4 heads/core).
Each core computes QKV projections for its heads, RoPE, causal
attention, and a partial output projection (its heads' rows of Wo);
the host sums the 4 partials per batch.

Layout strategy (no on-device transposes of activations):
 - host uploads xT = x[b].T; Q/K produced transposed [d, l]; V natural
   [l, d]; scores computed transposed ST[k, q]; exp(ST) in [k, q] is
   directly the moving operand of the AV matmul with V as stationary,
   giving UT[d, q] - exactly the Wo-matmul stationary layout.
 - softmax without max subtraction (scores bounded, exp in bf16 whose
   range covers e^60). Row sums via matmuls with an ALL-ONES [128,128]
   stationary, accumulated in PSUM alongside AV: the sum lands
   replicated across all 128 PSUM partitions, so 1/r needs no
   partition broadcast - one DVE reciprocal + one DVE multiply
   normalizes UT during eviction. Full-width k-tiles are summed in
   bf16 pairs then quads on DVE so they need only one row-sum
   matmul per 4 k-tiles; diagonal (causal-masked) tiles keep per-tile
   row sums.
 - causal diagonal blocks are trapezoids: ST/AV/row-sum matmuls
   restrict the moving operand to valid q >= 128*j, and exp+mask cover
   only that range. All 4 diagonal masks are windows of ONE [128,512]
   tril tile (mask[j][k,q]=(k<=q-128j) = mask0[k,q-128j]); the mask
   multiply runs on gpsimd (SBUF-only) to keep DVE off that chain.
 - RoPE rotate-half folded into the sin multiply: with the host-side
   even/odd deinterleave of Wq/Wk rows, rot(x)=[-x_odd; x_even] is a
   +-64-partition shift, done via DVE ops whose OUTPUT partition base
   differs from the (equal-base) input partitions - legal on HW (the
   verifier only requires the two SBUF inputs to share a base). The
   sin table's upper half is negated on device, so per strip RoPE is
   4 DVE ops reading the projection PSUM directly (no scalar
   eviction, no rotate matmul, no permutation constant).
 - cos/sin stream as [64, L] halves; the duplicated cos upper half
   and negated sin upper half are derived on DVE at startup. GpSimd
   is compute-useless here (~18 G elem/s, ~1us op overhead, no PSUM
   access, and its in-order queue delays the DMA issues behind it);
   it only issues wk/wv DMA descriptors.

Dtypes: Q/K side (x, Wq, Wk, Wv, cos/sin, Q, K) in fp16; P/V side
(exp, V, at, Wo) in bf16 (exp needs bf16 range); PSUM f32. Measured
end-to-end error ~2.5e-3 vs the 2e-2 gate. fp16/bf16 matmuls run at
the same PE rate as f32r but halve DMA and SBUF, so ALL weights are
SBUF-resident, loaded once at startup across three DGE queues
(scalar/gpsimd/sync) staggered behind c0's x stream; after chunk 0
the kernel streams only x (2MB/chunk, host pre-tiled to [c][128,
KT*CHUNK] so each chunk is ONE full-bandwidth DMA prefetched during
the previous chunk's attention) and is never DMA-paced. Chunk 0's
ramp is aggregate-HBM-bound (~6.6MB of x+weights before qk can
finish).

Perf structure:
 - attention processes heads in pairs with a one-iteration skew
   between the ST-matmul/exp stage and the AV/rowsum stage, so the
   tensor engine never stalls on the exp latency (stalls reset the PE
   p-state ramp: 1.2GHz for 3us after every gap, 2.4GHz after 3us of
   continuous execution).
 - PSUM: 3 "big" banks (proj accumulators / UT / out-proj) + 3 "st"
   banks (score tiles / out-proj) + 2 "rb" banks (row sums) = 8.
 - out-proj evictions alternate DVE/scalar and banks alternate
   big/st (6-deep rotation); stores alternate sync/scalar DGE queues;
   output partials stored bf16 (host sums in f32). The first three
   out-proj tiles compute their pair-0 half as complete PSUM groups
   in the then-idle "st" banks so the tensor engine has work while
   pair-1's normalization chain lands.

Hardware pitfall found on the way: splitting one PSUM accumulation
group's matmuls into two rounds with other start=True matmuls to
OTHER banks interleaved between them corrupts results on HW (CoreSim
accepts it); keep each tile's accumulation contiguous per bank.
GpSimd (Pool) cannot access PSUM (verifier NCC_IBIR); SBUF-SBUF only.
"""
import sys
import numpy as np

sys.path.insert(0, '/opt/trn_rl_repo')

import concourse.bass as bass  # noqa: E402,F401
import concourse.mybir as mybir  # noqa: E402
import concourse.tile as tile  # noqa: E402
from concourse import bacc  # noqa: E402
from concourse import library_config  # noqa: E402
from concourse.bass_utils import run_bass_kernel_spmd  # noqa: E402

B, L, D = 2, 2048, 2048
H, DH = 16, 128
HG = 4           # heads per core
G = H // HG      # head groups (cores per batch)
NCORES = 8
CHUNK = 512      # l-chunk
NCH = L // CHUNK          # 4 chunks
KT = D // 128             # 16 k-tiles over D
LT = L // 128             # 16 l-tiles
ROPE_BASE = 10000.0

f32 = mybir.dt.float32
f32r = mybir.dt.float32r
f16 = mybir.dt.float16
bf16 = mybir.dt.bfloat16

_built = None
PHASES = []


def _stage_weight_loads(nc, kt, wq_t, wq_d, wk_t, wk_d, wv_t, wv_d,
                        cos_t, cos_d, sin_t, sin_d, masks_t, mask_d,
                        ones_c, ones_c_d):
    """One-time weight/constant loads staggered behind c0's x stream.

    ALL weights ride the scalar DGE queue in exact consumption order
    (wq0, cos, sin, wq1-3, wk0-3); queue order itself paces them so
    nothing steals HBM bandwidth from the stream that gates the next
    strip. x quarters + late constants (wv, masks, ones) ride sync.
    """
    if kt == 3:
        nc.scalar.dma_start(out=wq_t[2][:], in_=wq_d[2])
        nc.scalar.dma_start(out=wq_t[3][:], in_=wq_d[3])
    elif kt == 7:
        nc.scalar.dma_start(out=wk_t[0][:], in_=wk_d[0])
        nc.scalar.dma_start(out=wk_t[1][:], in_=wk_d[1])
        nc.scalar.dma_start(out=wk_t[2][:], in_=wk_d[2])
        nc.scalar.dma_start(out=wk_t[3][:], in_=wk_d[3])
    elif kt == 15:
        # sync queue: behind all four x quarters by construction
        nc.sync.dma_start(out=wv_t[:, :8], in_=wv_d[:8].rearrange("k p f -> p k f"))
        nc.sync.dma_start(out=wv_t[:, 8:], in_=wv_d[8:].rearrange("k p f -> p k f"))
        nc.sync.dma_start(out=masks_t[:], in_=mask_d[:])
        nc.sync.dma_start(out=ones_c[:], in_=ones_c_d[:])


def _build():
    nc = bacc.Bacc()

    # xt: [c][p][kt*CHUNK+n] = x[b].T[kt*128+p, c*CHUNK+n] (host pre-tiled
    # so every DMA line is >=4KB contiguous per partition)
    xt_d = nc.declare_dram_parameter("xt", [NCH, 128, KT * CHUNK], f16,
                                     isOutput=False)
    # wq/wk: [m][p][kt*128+f] = W^T[kt*128+p, m*128+f]
    wq_d = nc.declare_dram_parameter("wq", [HG, 128, KT * 128], f16, isOutput=False)
    wk_d = nc.declare_dram_parameter("wk", [HG, 128, KT * 128], f16, isOutput=False)
    # wv: [kt][p][f] = Wv^T[kt*128+p, f]
    wv_d = nc.declare_dram_parameter("wv", [KT, 128, HG * 128], f16, isOutput=False)
    wo_d = nc.declare_dram_parameter("wo", [HG, 128, D], bf16, isOutput=False)
    cos_d = nc.declare_dram_parameter("cosT", [64, L], f16, isOutput=False)
    sin_d = nc.declare_dram_parameter("sinT", [64, L], f16, isOutput=False)
    mask_d = nc.declare_dram_parameter("masks", [128, CHUNK], bf16, isOutput=False)
    ones_c_d = nc.declare_dram_parameter("ones_c", [128, 128], bf16, isOutput=False)

    out_d = nc.declare_dram_parameter("out", [L, D], bf16, isOutput=True)

    with tile.TileContext(nc) as tc:
        with (
            tc.tile_pool(name="const", bufs=1) as const,
            tc.tile_pool(name="persist", bufs=1) as persist,
            tc.tile_pool(name="xs", bufs=2) as xs,            # flat x tiles
            tc.tile_pool(name="chact", bufs=4) as chact,      # per-chunk qt/at
            tc.tile_pool(name="tmps", bufs=2) as tmps,        # transients
            tc.tile_pool(name="etp", bufs=6) as etp,          # exp tiles (bf16)
            tc.tile_pool(name="small", bufs=2) as small,      # [1,512] tiles
            tc.tile_pool(name="ps", bufs=1, space="PSUM") as pp,
        ):
            # ---- resident weights ----
            wq_t = [persist.tile([128, KT * 128], f16, name=f"wqt{m}")
                    for m in range(HG)]
            wk_t = [persist.tile([128, KT * 128], f16, name=f"wkt{m}")
                    for m in range(HG)]
            wv_t = persist.tile([128, KT, HG * 128], f16, name="wvt")
            wo_t = [persist.tile([128, D], bf16, name=f"wot{h}") for h in range(HG)]
            # ---- constants ----
            cos_t = const.tile([128, L], f16)
            sin_t = const.tile([128, L], f16)
            masks_t = const.tile([128, CHUNK], bf16)
            ones_c = const.tile([128, 128], bf16)

            # scalar queue in consumption order: wq0 gates the first
            # strip; cos/sin gate its rope; then wq1-3 (wk0-3 staggered
            # later). DVE derives the duplicated cos / negated sin upper
            # halves well before the first strip's rope ops need them.
            nc.scalar.dma_start(out=wq_t[0][:], in_=wq_d[0])
            nc.scalar.dma_start(out=wq_t[1][:], in_=wq_d[1])
            nc.scalar.dma_start(out=cos_t[0:64, :], in_=cos_d[:])
            nc.scalar.dma_start(out=sin_t[0:64, :], in_=sin_d[:])
            nc.vector.tensor_copy(out=cos_t[64:128, :], in_=cos_t[0:64, :])
            nc.vector.tensor_scalar(out=sin_t[64:128, :], in0=sin_t[0:64, :],
                                    scalar1=-1.0, scalar2=None,
                                    op0=mybir.AluOpType.mult)

            # ---- persistent activations (full history) ----
            kt_t = [persist.tile([128, L], f16, name=f"ktt{h}") for h in range(HG)]
            v_t = [persist.tile([128, HG * 128], bf16, name=f"vt{lt}")
                   for lt in range(LT)]

            xf_next = None
            for c in range(NCH):
                PHASES.append((f"c{c}_load", int(nc.next_id())))
                cs = slice(c * CHUNK, (c + 1) * CHUNK)
                # ---------- x for chunk c (flat pre-tiled, 4KB lines) ------
                if c == 0:
                    xf = xs.tile([128, KT * CHUNK], f16, tag="xf", name="xf0")
                    for q in range(4):
                        nc.sync.dma_start(
                            out=xf[:, q * 4 * CHUNK:(q + 1) * 4 * CHUNK],
                            in_=xt_d[0, :, q * 4 * CHUNK:(q + 1) * 4 * CHUNK])
                        _stage_weight_loads(nc, 4 * q + 3, wq_t, wq_d, wk_t,
                                            wk_d, wv_t, wv_d, cos_t, cos_d,
                                            sin_t, sin_d, masks_t, mask_d,
                                            ones_c, ones_c_d)
                else:
                    xf = xf_next
                xc = [xf[:, kt * CHUNK:(kt + 1) * CHUNK] for kt in range(KT)]

                PHASES.append((f"c{c}_qk", int(nc.next_id())))
                # ---------- Q/K projections + RoPE ----------
                qt_c = [chact.tile([128, CHUNK], f16, tag="qtc", name=f"qtc{h}")
                        for h in range(HG)]
                for (w_t_, isq) in ((wq_t, True), (wk_t, False)):
                    for m in range(HG):
                        wm = w_t_[m]
                        ps = pp.tile([128, CHUNK], f32, tag="big", bufs=3)
                        for kt in range(KT):
                            nc.tensor.matmul(ps[:], wm[:, kt * 128:(kt + 1) * 128],
                                             xc[kt][:],
                                             start=(kt == 0), stop=(kt == KT - 1))
                        # RoPE fold: out = raw*cos + shift64(raw)*(+-sin).
                        # t2's DVE ops shift the OUTPUT partition base; the
                        # sin table's upper half holds -sin.
                        t1 = tmps.tile([128, CHUNK], f16, tag="t1")
                        nc.vector.tensor_tensor(out=t1[:], in0=ps[:],
                                                in1=cos_t[:, cs],
                                                op=mybir.AluOpType.mult)
                        t2 = tmps.tile([128, CHUNK], f16, tag="t2")
                        nc.vector.tensor_tensor(out=t2[0:64, :],
                                                in0=ps[64:128, :],
                                                in1=sin_t[64:128, cs],
                                                op=mybir.AluOpType.mult)
                        nc.vector.tensor_tensor(out=t2[64:128, :],
                                                in0=ps[0:64, :],
                                                in1=sin_t[0:64, cs],
                                                op=mybir.AluOpType.mult)
                        dst = qt_c[m] if isq else kt_t[m]
                        dst_ap = dst[:] if isq else dst[:, cs]
                        nc.vector.tensor_tensor(out=dst_ap, in0=t1[:], in1=t2[:],
                                                op=mybir.AluOpType.add)

                nkt = (c + 1) * (CHUNK // 128)   # causal: k-tiles 0..nkt-1
                ndiag = nkt - 4        # non-diag kts (multiple of 4)

                def emit_st_head(kt, h, et, hi):
                    """Score matmul + exp (+ mask) for one (kt, head) into
                    half ``hi`` of ``et``."""
                    diag_j = kt - ndiag
                    q0 = max(diag_j, 0) * 128   # trapezoid: valid q >= q0
                    st = pp.tile([128, CHUNK], f32, tag="st", bufs=3)
                    nc.tensor.matmul(
                        st[:, q0:], kt_t[h][:, kt * 128:(kt + 1) * 128],
                        qt_c[h][:, q0:], start=True, stop=True)
                    if diag_j >= 0:
                        eraw = etp.tile([128, CHUNK], bf16, tag="eraw",
                                        bufs=2)
                        nc.scalar.activation(
                            eraw[:, q0:], st[:, q0:],
                            mybir.ActivationFunctionType.Exp)
                        # tril window == mask for diag block j; one
                        # writer per et half (two-writer split races
                        # on HW - CoreSim does not catch it)
                        nc.vector.tensor_tensor(
                            out=et[:, hi, q0:], in0=eraw[:, q0:],
                            in1=masks_t[:, :CHUNK - q0],
                            op=mybir.AluOpType.mult)
                    else:
                        nc.scalar.activation(
                            et[:, hi, q0:], st[:, q0:],
                            mybir.ActivationFunctionType.Exp)
                    return q0

                def emit_st(kt, hs, tag="et"):
                    """Scores + exp for one kt, both heads of a pair;
                    returns the (et, q0) entry."""
                    # double-wide exp tile: [128, head, q]
                    et = etp.tile([128, 2, CHUNK], bf16, tag=tag)
                    for hi, h in enumerate(hs):
                        q0 = emit_st_head(kt, h, et, hi)
                    return (et, q0)

                PHASES.append((f"c{c}_v", int(nc.next_id())))
                # ---------- V projection ----------
                pre_et = None
                for sl in range(CHUNK // 128):
                    lt = c * (CHUNK // 128) + sl
                    if sl == CHUNK // 128 - 1:
                        # pair-0 kt=0 scores ahead of the last V strip: the
                        # exps land while V finishes, so attention starts
                        # with no warmup bubble
                        pre_et = emit_st(0, (0, 1))
                    ps = pp.tile([128, HG * 128], f32, tag="big", bufs=3)
                    for kt in range(KT):
                        nc.tensor.matmul(
                            ps[:], xc[kt][:, sl * 128:(sl + 1) * 128],
                            wv_t[:, kt, :],
                            start=(kt == 0), stop=(kt == KT - 1))
                    nc.vector.tensor_copy(out=v_t[lt][:], in_=ps[:])

                PHASES.append((f"c{c}_attn", int(nc.next_id())))
                if c + 1 < NCH:
                    xf_next = xs.tile([128, KT * CHUNK], f16, tag="xf",
                                      name=f"xf{c + 1}")
                    nc.sync.dma_start(out=xf_next[:], in_=xt_d[c + 1])
                # ---------- attention for q-chunk c (head pairs, skewed) ----
                at_c = [chact.tile([128, CHUNK], bf16, tag="atc", name=f"atc{h}")
                        for h in range(HG)]
                pre1 = {}
                for pair in range(2):
                    hs = (2 * pair, 2 * pair + 1)
                    ut = {h: pp.tile([128, CHUNK], f32, tag="big", bufs=3,
                                     name=f"ut{h}") for h in hs}
                    rs = {h: pp.tile([128, CHUNK], f32, tag="rb", bufs=2,
                                     name=f"rs{h}") for h in hs}
                    ets = {}
                    quads = {}
                    cums = {}
                    last_eps = None
                    for kt in range(nkt + 1):
                        if kt < nkt:
                            if pair == 0:
                                ets[kt] = pre_et if kt == 0 else emit_st(kt, hs)
                                if kt >= ndiag:
                                    # fill pair-0's chain-bound diagonal
                                    # region with pair-1's kt'=0/1 scores
                                    # (one ST per iteration keeps the
                                    # 3-bank st rotation stall-free)
                                    ktp, hh = divmod(kt - ndiag, 2)
                                    if hh == 0:
                                        pet = etp.tile([128, 2, CHUNK], bf16,
                                                       tag="pet", bufs=2)
                                        pre1[ktp] = (pet,
                                                     max(ktp - ndiag, 0) * 128)
                                    emit_st_head(ktp, 2 + hh,
                                                 pre1[ktp][0], hh)
                            else:
                                ets[kt] = (pre1[kt] if kt < 2
                                           else emit_st(kt, hs))
                            # bf16 pair/quad sums of full-width kts; one
                            # row-sum matmul per 4 kts
                            if kt % 2 == 1 and kt < ndiag:
                                eps = etp.tile([128, 2, CHUNK], bf16, tag="eps",
                                               bufs=3)
                                nc.vector.tensor_tensor(
                                    out=eps[:], in0=ets[kt - 1][0][:],
                                    in1=ets[kt][0][:], op=mybir.AluOpType.add)
                                if kt % 4 == 1:
                                    last_eps = eps
                                else:
                                    eqs = etp.tile([128, 2, CHUNK], bf16,
                                                   tag="eqs", bufs=2)
                                    nc.vector.tensor_tensor(
                                        out=eqs[:], in0=last_eps[:], in1=eps[:],
                                        op=mybir.AluOpType.add)
                                    quads[kt] = eqs
                        if kt >= 1:
                            e, eq0 = ets.pop(kt - 1)
                            first, last = kt - 1 == 0, kt - 1 == nkt - 1
                            for hi, h in enumerate(hs):
                                nc.tensor.matmul(
                                    ut[h][:, eq0:],
                                    v_t[kt - 1][:, h * 128:(h + 1) * 128],
                                    e[:, hi, eq0:],
                                    start=first, stop=last)
                            if kt - 1 < ndiag:
                                if (kt - 1) % 4 == 3:
                                    # row sums via the quad-sum
                                    eqs = quads.pop(kt - 1)
                                    for hi, h in enumerate(hs):
                                        nc.tensor.matmul(
                                            rs[h][:], ones_c[:],
                                            eqs[:, hi, :],
                                            start=(kt - 1 == 3), stop=False)
                            else:
                                for hi, h in enumerate(hs):
                                    nc.tensor.matmul(
                                        rs[h][:, eq0:], ones_c[:],
                                        e[:, hi, eq0:],
                                        start=first, stop=last)
                    for hi, h in enumerate(hs):
                        rb_sb = tmps.tile([128, CHUNK], f32, tag="bc", bufs=2)
                        nc.vector.reciprocal_approx_fast(out=rb_sb[:],
                                                         in_=rs[h][:])
                        nc.vector.tensor_tensor(out=at_c[h][:], in0=ut[h][:],
                                                in1=rb_sb[:],
                                                op=mybir.AluOpType.mult)

                PHASES.append((f"c{c}_out", int(nc.next_id())))
                # ---------- output projection for chunk c ----------
                if c == 0:
                    for h in range(HG):
                        nc.sync.dma_start(out=wo_t[h][:], in_=wo_d[h])
                # first five tiles: pair-0 halves as COMPLETE groups run
                # before pair-1's normalization chain (2 recip + 2 mult on
                # DVE, ~2.4us) lands; halves summed on DVE at eviction.
                # "st" banks are free during out-proj, so these half-groups
                # need no "big" bank (those are freed BY the chain).
                NHALF = 5
                half_a = {}
                for idx in range(NHALF):
                    ot, sl = divmod(idx, 4)
                    opa = pp.tile([128, 512], f32, tag="st", bufs=3,
                                  name=f"opa{idx}")
                    for h in (0, 1):
                        nc.tensor.matmul(
                            opa[:], at_c[h][:, sl * 128:(sl + 1) * 128],
                            wo_t[h][:, ot * 512:(ot + 1) * 512],
                            start=(h == 0), stop=(h == 1))
                    osa = tmps.tile([128, 512], f32, tag="osba", bufs=NHALF)
                    nc.scalar.copy(out=osa[:], in_=opa[:])
                    half_a[idx] = osa
                for idx in range(16):
                    ot, sl = divmod(idx, 4)
                    mt = c * (CHUNK // 128) + sl
                    osb = tmps.tile([128, 512], bf16, tag="osb", bufs=6)
                    if idx < NHALF:
                        opb = pp.tile([128, 512], f32, tag="big", bufs=3)
                        for h in (2, 3):
                            nc.tensor.matmul(
                                opb[:], at_c[h][:, sl * 128:(sl + 1) * 128],
                                wo_t[h][:, ot * 512:(ot + 1) * 512],
                                start=(h == 2), stop=(h == 3))
                        nc.vector.tensor_tensor(out=osb[:], in0=half_a[idx][:],
                                                in1=opb[:],
                                                op=mybir.AluOpType.add)
                    else:
                        # banks alternate big/st (6-deep rotation);
                        # evictions alternate DVE/scalar
                        ops = pp.tile([128, 512], f32,
                                      tag=("big" if idx % 2 else "st"), bufs=3)
                        for h in range(HG):
                            nc.tensor.matmul(
                                ops[:], at_c[h][:, sl * 128:(sl + 1) * 128],
                                wo_t[h][:, ot * 512:(ot + 1) * 512],
                                start=(h == 0), stop=(h == HG - 1))
                        if idx % 2:
                            nc.vector.tensor_copy(out=osb[:], in_=ops[:])
                        else:
                            nc.scalar.copy(out=osb[:], in_=ops[:])
                    qeng = nc.scalar if idx % 4 == 1 else nc.sync
                    qeng.dma_start(
                        out=out_d[mt * 128:(mt + 1) * 128, ot * 512:(ot + 1) * 512],
                        in_=osb[:])

    nc.finalize()
    return nc


def _get_nc():
    global _built
    if _built is None:
        _built = _build()
    return _built


def _host_prep(x, positions, Wq, Wk, Wv, Wo):
    """Build per-core input maps."""
    import ml_dtypes
    x = np.asarray(x, np.float32)
    positions = np.asarray(positions)
    Wq = np.asarray(Wq, np.float32)
    Wk = np.asarray(Wk, np.float32)
    Wv = np.asarray(Wv, np.float32)
    Wo = np.asarray(Wo, np.float32)

    scale = np.float32(1.0 / np.sqrt(DH))
    perm = np.concatenate([np.arange(0, DH, 2), np.arange(1, DH, 2)])  # deinterleave

    Wq_p = (Wq * scale).reshape(H, DH, D)[:, perm, :]   # [H, dh, D]
    Wk_p = Wk.reshape(H, DH, D)[:, perm, :]

    # RoPE tables per batch, [64, L] halves (device derives dup/neg)
    inv_freq = 1.0 / (ROPE_BASE ** (np.arange(0, DH, 2, dtype=np.float32) / DH))
    cosT = np.empty((B, 64, L), np.float32)
    sinT = np.empty((B, 64, L), np.float32)
    for b in range(B):
        freqs = positions[b].astype(np.float32)[:, None] * inv_freq[None, :]  # [L, 64]
        cosT[b] = np.cos(freqs).T.astype(np.float32)  # [64, L]
        sinT[b] = np.sin(freqs).T.astype(np.float32)

    # single tril tile; diagonal block j's mask is its width-(512-128j) window
    kk = np.arange(128)[:, None]
    qq = np.arange(CHUNK)[None, :]
    mask0 = (kk <= qq).astype(np.float32).astype(ml_dtypes.bfloat16)

    ones_c = np.ones((128, 128), ml_dtypes.bfloat16)

    in_maps = []
    for core in range(NCORES):
        b, g = divmod(core, G)
        hs = slice(g * HG, (g + 1) * HG)
        # W^T for this core's heads: [D, HG*dh]
        wqT = Wq_p[hs].reshape(HG * DH, D).T          # [D, 512]
        wkT = Wk_p[hs].reshape(HG * DH, D).T
        wvT = Wv.reshape(H, DH, D)[hs].reshape(HG * DH, D).T
        # [m][p][kt*128+f] layout
        wq_c = np.ascontiguousarray(
            wqT.reshape(KT, 128, HG, DH).transpose(2, 1, 0, 3).reshape(
                HG, 128, KT * DH)).astype(np.float16)
        wk_c = np.ascontiguousarray(
            wkT.reshape(KT, 128, HG, DH).transpose(2, 1, 0, 3).reshape(
                HG, 128, KT * DH)).astype(np.float16)
        # [kt][p][f]
        wv_c = np.ascontiguousarray(
            wvT.reshape(KT, 128, HG * DH)).astype(np.float16)
        # wo[h][d'][o] = Wo[o, (g*HG+h)*dh + d']
        wo_c = np.ascontiguousarray(
            Wo.T.reshape(H, DH, D)[hs]).astype(ml_dtypes.bfloat16)  # [HG, dh, D]
        xtb = x[b].T.astype(np.float16)   # [D, L]
        xt_tiled = np.ascontiguousarray(
            xtb.reshape(KT, 128, NCH, CHUNK).transpose(2, 1, 0, 3).reshape(
                NCH, 128, KT * CHUNK))
        in_maps.append({
            "xt": xt_tiled,
            "wq": wq_c, "wk": wk_c, "wv": wv_c, "wo": wo_c,
            "cosT": cosT[b].astype(np.float16),
            "sinT": sinT[b].astype(np.float16),
            "masks": mask0,
            "ones_c": ones_c,
        })
    return in_maps


def kernel(x, positions, Wq, Wk, Wv, Wo, _profile=False):
    nc = _get_nc()
    in_maps = _host_prep(x, positions, Wq, Wk, Wv, Wo)
    res = run_bass_kernel_spmd(nc, in_maps, list(range(NCORES)), trace=_profile)
    out = np.zeros((B, L, D), np.float32)
    for core in range(NCORES):
        b = core // G
        out[b] += np.asarray(res.results[core]["out"], np.float32)
    if _profile:
        kernel._last_exec_time_ns = res.exec_time_ns
        kernel._last_trace = res.instructions_and_trace
    return out


# revision 23
# speedup vs baseline: 1.0154x; 1.0002x over previous
"""Multi-head attention (RoPE, causal) Trainium2 kernel.

Problem: B=2, L=2048, D=2048, H=16, dh=128, fp32.
Sharding: 8 cores = 2 batches x 4 head-groups (4 heads/core).
Each core computes QKV projections for its heads, RoPE, causal
attention, and a partial output projection (its heads' rows of Wo);
the host sums the 4 partials per batch.

Layout strategy (no on-device transposes of activations):
 - host uploads xT = x[b].T; Q/K produced transposed [d, l]; V natural
   [l, d]; scores computed transposed ST[k, q]; exp(ST) in [k, q] is
   directly the moving operand of the AV matmul with V as stationary,
   giving UT[d, q] - exactly the Wo-matmul stationary layout.
 - softmax without max subtraction (scores bounded, exp in bf16 whose
   range covers e^60). Row sums via matmuls with an ALL-ONES [128,128]
   stationary, accumulated in PSUM alongside AV: the sum lands
   replicated across all 128 PSUM partitions, so 1/r needs no
   partition broadcast - one DVE reciprocal + one DVE multiply
   normalizes UT during eviction. Full-width k-tiles are summed in
   bf16 pairs then quads on DVE so they need only one row-sum
   matmul per 4 k-tiles; diagonal (causal-masked) tiles keep per-tile
   row sums.
 - causal diagonal blocks are trapezoids: ST/AV/row-sum matmuls
   restrict the moving operand to valid q >= 128*j, and exp+mask cover
   only that range. All 4 diagonal masks are windows of ONE [128,512]
   tril tile (mask[j][k,q]=(k<=q-128j) = mask0[k,q-128j]); the mask
   multiply runs on gpsimd (SBUF-only) to keep DVE off that chain.
 - RoPE rotate-half folded into the sin multiply: with the host-side
   even/odd deinterleave of Wq/Wk rows, rot(x)=[-x_odd; x_even] is a
   +-64-partition shift, done via DVE ops whose OUTPUT partition base
   differs from the (equal-base) input partitions - legal on HW (the
   verifier only requires the two SBUF inputs to share a base). The
   sin table's upper half is negated on device, so per strip RoPE is
   4 DVE ops reading the projection PSUM directly (no scalar
   eviction, no rotate matmul, no permutation constant).
 - cos/sin stream as [64, L] halves; the duplicated cos upper half
   and negated sin upper half are derived on DVE at startup. GpSimd
   is compute-useless here (~18 G elem/s, ~1us op overhead, no PSUM
   access, and its in-order queue delays the DMA issues behind it);
   it only issues wk/wv DMA descriptors.

Dtypes: Q/K side (x, Wq, Wk, Wv, cos/sin, Q, K) in fp16; P/V side
(exp, V, at, Wo) in bf16 (exp needs bf16 range); PSUM f32. Measured
end-to-end error ~2.5e-3 vs the 2e-2 gate. fp16/bf16 matmuls run at
the same PE rate as f32r but halve DMA and SBUF, so ALL weights are
SBUF-resident, loaded once at startup across three DGE queues
(scalar/gpsimd/sync) staggered behind c0's x stream; after chunk 0
the kernel streams only x (2MB/chunk, host pre-tiled to [c][128,
KT*CHUNK] so each chunk is ONE full-bandwidth DMA prefetched during
the previous chunk's attention) and is never DMA-paced. Chunk 0's
ramp is aggregate-HBM-bound (~6.6MB of x+weights before qk can
finish).

Perf structure:
 - attention processes heads in pairs with a one-iteration skew
   between the ST-matmul/exp stage and the AV/rowsum stage, so the
   tensor engine never stalls on the exp latency (stalls reset the PE
   p-state ramp: 1.2GHz for 3us after every gap, 2.4GHz after 3us of
   continuous execution).
 - PSUM: 3 "big" banks (proj accumulators / UT / out-proj) + 3 "st"
   banks (score tiles / out-proj) + 2 "rb" banks (row sums) = 8.
 - out-proj evictions alternate DVE/scalar and banks alternate
   big/st (6-deep rotation); stores alternate sync/scalar DGE queues;
   output partials stored bf16 (host sums in f32). The first three
   out-proj tiles compute their pair-0 half as complete PSUM groups
   in the then-idle "st" banks so the tensor engine has work while
   pair-1's normalization chain lands.

Hardware pitfall found on the way: splitting one PSUM accumulation
group's matmuls into two rounds with other start=True matmuls to
OTHER banks interleaved between them corrupts results on HW (CoreSim
accepts it); keep each tile's accumulation contiguous per bank.
GpSimd (Pool) cannot access PSUM (verifier NCC_IBIR); SBUF-SBUF only.
"""
import sys
import numpy as np

sys.path.insert(0, '/opt/trn_rl_repo')

import concourse.bass as bass  # noqa: E402,F401
import concourse.mybir as mybir  # noqa: E402
import concourse.tile as tile  # noqa: E402
from concourse import bacc  # noqa: E402
from concourse import library_config  # noqa: E402
from concourse.bass_utils import run_bass_kernel_spmd  # noqa: E402

B, L, D = 2, 2048, 2048
H, DH = 16, 128
HG = 4           # heads per core
G = H // HG      # head groups (cores per batch)
NCORES = 8
CHUNK = 512      # l-chunk
NCH = L // CHUNK          # 4 chunks
KT = D // 128             # 16 k-tiles over D
LT = L // 128             # 16 l-tiles
ROPE_BASE = 10000.0

f32 = mybir.dt.float32
f32r = mybir.dt.float32r
f16 = mybir.dt.float16
bf16 = mybir.dt.bfloat16

_built = None
PHASES = []


def _stage_weight_loads(nc, kt, wq_t, wq_d, wk_t, wk_d, wv_t, wv_d,
                        cos_t, cos_d, sin_t, sin_d, masks_t, mask_d,
                        ones_c, ones_c_d):
    """One-time weight/constant loads staggered behind c0's x stream.

    ALL weights ride the scalar DGE queue in exact consumption order
    (wq0, cos, sin, wq1-3, wk0-3); queue order itself paces them so
    nothing steals HBM bandwidth from the stream that gates the next
    strip. x quarters + late constants (wv, masks, ones) ride sync.
    """
    if kt == 3:
        nc.scalar.dma_start(out=wq_t[2][:], in_=wq_d[2])
        nc.scalar.dma_start(out=wq_t[3][:], in_=wq_d[3])
    elif kt == 7:
        nc.scalar.dma_start(out=wk_t[0][:], in_=wk_d[0])
        nc.scalar.dma_start(out=wk_t[1][:], in_=wk_d[1])
        nc.scalar.dma_start(out=wk_t[2][:], in_=wk_d[2])
        nc.scalar.dma_start(out=wk_t[3][:], in_=wk_d[3])
    elif kt == 15:
        # sync queue: behind all four x quarters by construction
        nc.sync.dma_start(out=wv_t[:, :8], in_=wv_d[:8].rearrange("k p f -> p k f"))
        nc.sync.dma_start(out=wv_t[:, 8:], in_=wv_d[8:].rearrange("k p f -> p k f"))
        nc.sync.dma_start(out=masks_t[:], in_=mask_d[:])
        nc.sync.dma_start(out=ones_c[:], in_=ones_c_d[:])


def _build():
    nc = bacc.Bacc()

    # xt: [c][p][kt*CHUNK+n] = x[b].T[kt*128+p, c*CHUNK+n] (host pre-tiled
    # so every DMA line is >=4KB contiguous per partition)
    xt_d = nc.declare_dram_parameter("xt", [NCH, 128, KT * CHUNK], f16,
                                     isOutput=False)
    # wq/wk: [m][p][kt*128+f] = W^T[kt*128+p, m*128+f]
    wq_d = nc.declare_dram_parameter("wq", [HG, 128, KT * 128], f16, isOutput=False)
    wk_d = nc.declare_dram_parameter("wk", [HG, 128, KT * 128], f16, isOutput=False)
    # wv: [kt][p][f] = Wv^T[kt*128+p, f]
    wv_d = nc.declare_dram_parameter("wv", [KT, 128, HG * 128], f16, isOutput=False)
    wo_d = nc.declare_dram_parameter("wo", [HG, 128, D], bf16, isOutput=False)
    cos_d = nc.declare_dram_parameter("cosT", [64, L], f16, isOutput=False)
    sin_d = nc.declare_dram_parameter("sinT", [64, L], f16, isOutput=False)
    mask_d = nc.declare_dram_parameter("masks", [128, CHUNK], bf16, isOutput=False)
    ones_c_d = nc.declare_dram_parameter("ones_c", [128, 128], bf16, isOutput=False)

    out_d = nc.declare_dram_parameter("out", [L, D], bf16, isOutput=True)

    with tile.TileContext(nc) as tc:
        with (
            tc.tile_pool(name="const", bufs=1) as const,
            tc.tile_pool(name="persist", bufs=1) as persist,
            tc.tile_pool(name="xs", bufs=2) as xs,            # flat x tiles
            tc.tile_pool(name="chact", bufs=4) as chact,      # per-chunk qt/at
            tc.tile_pool(name="tmps", bufs=2) as tmps,        # transients
            tc.tile_pool(name="etp", bufs=6) as etp,          # exp tiles (bf16)
            tc.tile_pool(name="small", bufs=2) as small,      # [1,512] tiles
            tc.tile_pool(name="ps", bufs=1, space="PSUM") as pp,
        ):
            # ---- resident weights ----
            wq_t = [persist.tile([128, KT * 128], f16, name=f"wqt{m}")
                    for m in range(HG)]
            wk_t = [persist.tile([128, KT * 128], f16, name=f"wkt{m}")
                    for m in range(HG)]
            wv_t = persist.tile([128, KT, HG * 128], f16, name="wvt")
            wo_t = [persist.tile([128, D], bf16, name=f"wot{h}") for h in range(HG)]
            # ---- constants ----
            cos_t = const.tile([128, L], f16)
            sin_t = const.tile([128, L], f16)
            masks_t = const.tile([128, CHUNK], bf16)
            ones_c = const.tile([128, 128], bf16)

            # scalar queue in consumption order: wq0 gates the first
            # strip; cos/sin gate its rope; then wq1-3 (wk0-3 staggered
            # later). DVE derives the duplicated cos / negated sin upper
            # halves well before the first strip's rope ops need them.
            nc.scalar.dma_start(out=wq_t[0][:], in_=wq_d[0])
            nc.scalar.dma_start(out=wq_t[1][:], in_=wq_d[1])
            nc.scalar.dma_start(out=cos_t[0:64, :], in_=cos_d[:])
            nc.scalar.dma_start(out=sin_t[0:64, :], in_=sin_d[:])
            nc.vector.tensor_copy(out=cos_t[64:128, :], in_=cos_t[0:64, :])
            nc.vector.tensor_scalar(out=sin_t[64:128, :], in0=sin_t[0:64, :],
                                    scalar1=-1.0, scalar2=None,
                                    op0=mybir.AluOpType.mult)

            # ---- persistent activations (full history) ----
            kt_t = [persist.tile([128, L], f16, name=f"ktt{h}") for h in range(HG)]
            v_t = [persist.tile([128, HG * 128], bf16, name=f"vt{lt}")
                   for lt in range(LT)]

            xf_next = None
            for c in range(NCH):
                PHASES.append((f"c{c}_load", int(nc.next_id())))
                cs = slice(c * CHUNK, (c + 1) * CHUNK)
                # ---------- x for chunk c (flat pre-tiled, 4KB lines) ------
                if c == 0:
                    xf = xs.tile([128, KT * CHUNK], f16, tag="xf", name="xf0")
                    for q in range(4):
                        nc.sync.dma_start(
                            out=xf[:, q * 4 * CHUNK:(q + 1) * 4 * CHUNK],
                            in_=xt_d[0, :, q * 4 * CHUNK:(q + 1) * 4 * CHUNK])
                        _stage_weight_loads(nc, 4 * q + 3, wq_t, wq_d, wk_t,
                                            wk_d, wv_t, wv_d, cos_t, cos_d,
                                            sin_t, sin_d, masks_t, mask_d,
                                            ones_c, ones_c_d)
                else:
                    xf = xf_next
                xc = [xf[:, kt * CHUNK:(kt + 1) * CHUNK] for kt in range(KT)]

                PHASES.append((f"c{c}_qk", int(nc.next_id())))
                # ---------- Q/K projections + RoPE ----------
                qt_c = [chact.tile([128, CHUNK], f16, tag="qtc", name=f"qtc{h}")
                        for h in range(HG)]
                for (w_t_, isq) in ((wq_t, True), (wk_t, False)):
                    for m in range(HG):
                        wm = w_t_[m]
                        ps = pp.tile([128, CHUNK], f32, tag="big", bufs=3)
                        for kt in range(KT):
                            nc.tensor.matmul(ps[:], wm[:, kt * 128:(kt + 1) * 128],
                                             xc[kt][:],
                                             start=(kt == 0), stop=(kt == KT - 1))
                        # RoPE fold: out = raw*cos + shift64(raw)*(+-sin).
                        # t2's DVE ops shift the OUTPUT partition base; the
                        # sin table's upper half holds -sin.
                        t1 = tmps.tile([128, CHUNK], f16, tag="t1")
                        nc.vector.tensor_tensor(out=t1[:], in0=ps[:],
                                                in1=cos_t[:, cs],
                                                op=mybir.AluOpType.mult)
                        t2 = tmps.tile([128, CHUNK], f16, tag="t2")
                        nc.vector.tensor_tensor(out=t2[0:64, :],
                                                in0=ps[64:128, :],
                                                in1=sin_t[64:128, cs],
                                                op=mybir.AluOpType.mult)
                        nc.vector.tensor_tensor(out=t2[64:128, :],
                                                in0=ps[0:64, :],
                                                in1=sin_t[0:64, cs],
                                                op=mybir.AluOpType.mult)
                        dst = qt_c[m] if isq else kt_t[m]
                        dst_ap = dst[:] if isq else dst[:, cs]
                        nc.vector.tensor_tensor(out=dst_ap, in0=t1[:], in1=t2[:],
                                                op=mybir.AluOpType.add)

                nkt = (c + 1) * (CHUNK // 128)   # causal: k-tiles 0..nkt-1
                ndiag = nkt - 4        # non-diag kts (multiple of 4)

                def emit_st_head(kt, h, et, hi):
                    """Score matmul + exp (+ mask) for one (kt, head) into
                    half ``hi`` of ``et``."""
                    diag_j = kt - ndiag
                    q0 = max(diag_j, 0) * 128   # trapezoid: valid q >= q0
                    st = pp.tile([128, CHUNK], f32, tag="st", bufs=3)
                    nc.tensor.matmul(
                        st[:, q0:], kt_t[h][:, kt * 128:(kt + 1) * 128],
                        qt_c[h][:, q0:], start=True, stop=True)
                    if diag_j >= 0:
                        eraw = etp.tile([128, CHUNK], bf16, tag="eraw",
                                        bufs=2)
                        nc.scalar.activation(
                            eraw[:, q0:], st[:, q0:],
                            mybir.ActivationFunctionType.Exp)
                        # tril window == mask for diag block j; one
                        # writer per et half (two-writer split races
                        # on HW - CoreSim does not catch it)
                        nc.vector.tensor_tensor(
                            out=et[:, hi, q0:], in0=eraw[:, q0:],
                            in1=masks_t[:, :CHUNK - q0],
                            op=mybir.AluOpType.mult)
                    else:
                        nc.scalar.activation(
                            et[:, hi, q0:], st[:, q0:],
                            mybir.ActivationFunctionType.Exp)
                    return q0

                def emit_st(kt, hs, tag="et"):
                    """Scores + exp for one kt, both heads of a pair;
                    returns the (et, q0) entry."""
                    # double-wide exp tile: [128, head, q]
                    et = etp.tile([128, 2, CHUNK], bf16, tag=tag)
                    for hi, h in enumerate(hs):
                        q0 = emit_st_head(kt, h, et, hi)
                    return (et, q0)

                def emit_v_strip(xsrc, cc, sl):
                    ps = pp.tile([128, HG * 128], f32, tag="big", bufs=3)
                    for kt in range(KT):
                        nc.tensor.matmul(
                            ps[:], xsrc[kt][:, sl * 128:(sl + 1) * 128],
                            wv_t[:, kt, :],
                            start=(kt == 0), stop=(kt == KT - 1))
                    nc.vector.tensor_copy(
                        out=v_t[cc * (CHUNK // 128) + sl][:], in_=ps[:])

                PHASES.append((f"c{c}_v", int(nc.next_id())))
                # ---------- V projection ----------
                # sl=0 of chunks 2/3 was computed inside the previous
                # chunk's attention (dense filler for its chain bubbles)
                pre_et = None
                for sl in range((1 if c >= 2 else 0), CHUNK // 128):
                    if sl == CHUNK // 128 - 1:
                        # pair-0 kt=0 scores ahead of the last V strip: the
                        # exps land while V finishes, so attention starts
                        # with no warmup bubble
                        pre_et = emit_st(0, (0, 1))
                    emit_v_strip(xc, c, sl)

                PHASES.append((f"c{c}_attn", int(nc.next_id())))
                if c + 1 < NCH:
                    xf_next = xs.tile([128, KT * CHUNK], f16, tag="xf",
                                      name=f"xf{c + 1}")
                    nc.sync.dma_start(out=xf_next[:], in_=xt_d[c + 1])
                # ---------- attention for q-chunk c (head pairs, skewed) ----
                at_c = [chact.tile([128, CHUNK], bf16, tag="atc", name=f"atc{h}")
                        for h in range(HG)]
                pre1 = {}
                for pair in range(2):
                    hs = (2 * pair, 2 * pair + 1)
                    ut = {h: pp.tile([128, CHUNK], f32, tag="big", bufs=3,
                                     name=f"ut{h}") for h in hs}
                    rs = {h: pp.tile([128, CHUNK], f32, tag="rb", bufs=2,
                                     name=f"rs{h}") for h in hs}
                    ets = {}
                    quads = {}
                    cums = {}
                    last_eps = None
                    for kt in range(nkt + 1):
                        if kt < nkt:
                            if pair == 0:
                                ets[kt] = pre_et if kt == 0 else emit_st(kt, hs)
                                if kt >= ndiag:
                                    # fill pair-0's chain-bound diagonal
                                    # region with pair-1's kt'=0/1 scores
                                    # (one ST per iteration keeps the
                                    # 3-bank st rotation stall-free)
                                    ktp, hh = divmod(kt - ndiag, 2)
                                    if hh == 0:
                                        pet = etp.tile([128, 2, CHUNK], bf16,
                                                       tag="pet", bufs=2)
                                        pre1[ktp] = (pet,
                                                     max(ktp - ndiag, 0) * 128)
                                    emit_st_head(ktp, 2 + hh,
                                                 pre1[ktp][0], hh)
                            else:
                                ets[kt] = (pre1[kt] if kt < 2
                                           else emit_st(kt, hs))
                                if kt == ndiag + 1 and c in (1, 2):
                                    # dense 16-matmul filler in the diag
                                    # region's latency shadow: next chunk's
                                    # first V strip (x prefetched, weights
                                    # resident, 3rd big bank free here)
                                    xcn = [xf_next[:, k * CHUNK:(k + 1) * CHUNK]
                                           for k in range(KT)]
                                    emit_v_strip(xcn, c + 1, 0)
                            # bf16 pair/quad sums of full-width kts; one
                            # row-sum matmul per 4 kts
                            if kt % 2 == 1 and kt < ndiag:
                                eps = etp.tile([128, 2, CHUNK], bf16, tag="eps",
                                               bufs=3)
                                nc.vector.tensor_tensor(
                                    out=eps[:], in0=ets[kt - 1][0][:],
                                    in1=ets[kt][0][:], op=mybir.AluOpType.add)
                                if kt % 4 == 1:
                                    last_eps = eps
                                else:
                                    eqs = etp.tile([128, 2, CHUNK], bf16,
                                                   tag="eqs", bufs=2)
                                    nc.vector.tensor_tensor(
                                        out=eqs[:], in0=last_eps[:], in1=eps[:],
                                        op=mybir.AluOpType.add)
                                    quads[kt] = eqs
                        if kt >= 1:
                            e, eq0 = ets.pop(kt - 1)
                            first, last = kt - 1 == 0, kt - 1 == nkt - 1
                            for hi, h in enumerate(hs):
                                nc.tensor.matmul(
                                    ut[h][:, eq0:],
                                    v_t[kt - 1][:, h * 128:(h + 1) * 128],
                                    e[:, hi, eq0:],
                                    start=first, stop=last)
                            if kt - 1 < ndiag:
                                if (kt - 1) % 4 == 3:
                                    # row sums via the quad-sum
                                    eqs = quads.pop(kt - 1)
                                    for hi, h in enumerate(hs):
                                        nc.tensor.matmul(
                                            rs[h][:], ones_c[:],
                                            eqs[:, hi, :],
                                            start=(kt - 1 == 3), stop=False)
                            else:
                                for hi, h in enumerate(hs):
                                    nc.tensor.matmul(
                                        rs[h][:, eq0:], ones_c[:],
                                        e[:, hi, eq0:],
                                        start=first, stop=last)
                    for hi, h in enumerate(hs):
                        rb_sb = tmps.tile([128, CHUNK], f32, tag="bc", bufs=2)
                        nc.vector.reciprocal_approx_fast(out=rb_sb[:],
                                                         in_=rs[h][:])
                        nc.vector.tensor_tensor(out=at_c[h][:], in0=ut[h][:],
                                                in1=rb_sb[:],
                                                op=mybir.AluOpType.mult)

                PHASES.append((f"c{c}_out", int(nc.next_id())))
                # ---------- output projection for chunk c ----------
                if c == 0:
                    for h in range(HG):
                        nc.sync.dma_start(out=wo_t[h][:], in_=wo_d[h])
                # first five tiles: pair-0 halves as COMPLETE groups run
                # before pair-1's normalization chain (2 recip + 2 mult on
                # DVE, ~2.4us) lands; halves summed on DVE at eviction.
                # "st" banks are free during out-proj, so these half-groups
                # need no "big" bank (those are freed BY the chain).
                NHALF = 5
                half_a = {}
                for idx in range(NHALF):
                    ot, sl = divmod(idx, 4)
                    opa = pp.tile([128, 512], f32, tag="st", bufs=3,
                                  name=f"opa{idx}")
                    for h in (0, 1):
                        nc.tensor.matmul(
                            opa[:], at_c[h][:, sl * 128:(sl + 1) * 128],
                            wo_t[h][:, ot * 512:(ot + 1) * 512],
                            start=(h == 0), stop=(h == 1))
                    osa = tmps.tile([128, 512], f32, tag="osba", bufs=NHALF)
                    nc.scalar.copy(out=osa[:], in_=opa[:])
                    half_a[idx] = osa
                for idx in range(16):
                    ot, sl = divmod(idx, 4)
                    mt = c * (CHUNK // 128) + sl
                    osb = tmps.tile([128, 512], bf16, tag="osb", bufs=6)
                    if idx < NHALF:
                        opb = pp.tile([128, 512], f32, tag="big", bufs=3)
                        for h in (2, 3):
                            nc.tensor.matmul(
                                opb[:], at_c[h][:, sl * 128:(sl + 1) * 128],
                                wo_t[h][:, ot * 512:(ot + 1) * 512],
                                start=(h == 2), stop=(h == 3))
                        nc.vector.tensor_tensor(out=osb[:], in0=half_a[idx][:],
                                                in1=opb[:],
                                                op=mybir.AluOpType.add)
                    else:
                        # banks alternate big/st (6-deep rotation);
                        # evictions alternate DVE/scalar
                        ops = pp.tile([128, 512], f32,
                                      tag=("big" if idx % 2 else "st"), bufs=3)
                        for h in range(HG):
                            nc.tensor.matmul(
                                ops[:], at_c[h][:, sl * 128:(sl + 1) * 128],
                                wo_t[h][:, ot * 512:(ot + 1) * 512],
                                start=(h == 0), stop=(h == HG - 1))
                        if idx % 2:
                            nc.vector.tensor_copy(out=osb[:], in_=ops[:])
                        else:
                            nc.scalar.copy(out=osb[:], in_=ops[:])
                    qeng = nc.scalar if idx % 4 == 1 else nc.sync
                    qeng.dma_start(
                        out=out_d[mt * 128:(mt + 1) * 128, ot * 512:(ot + 1) * 512],
                        in_=osb[:])

    nc.finalize()
    return nc


def _get_nc():
    global _built
    if _built is None:
        _built = _build()
    return _built


def _host_prep(x, positions, Wq, Wk, Wv, Wo):
    """Build per-core input maps."""
    import ml_dtypes
    x = np.asarray(x, np.float32)
    positions = np.asarray(positions)
    Wq = np.asarray(Wq, np.float32)
    Wk = np.asarray(Wk, np.float32)
    Wv = np.asarray(Wv, np.float32)
    Wo = np.asarray(Wo, np.float32)

    scale = np.float32(1.0 / np.sqrt(DH))
    perm = np.concatenate([np.arange(0, DH, 2), np.arange(1, DH, 2)])  # deinterleave

    Wq_p = (Wq * scale).reshape(H, DH, D)[:, perm, :]   # [H, dh, D]
    Wk_p = Wk.reshape(H, DH, D)[:, perm, :]

    # RoPE tables per batch, [64, L] halves (device derives dup/neg)
    inv_freq = 1.0 / (ROPE_BASE ** (np.arange(0, DH, 2, dtype=np.float32) / DH))
    cosT = np.empty((B, 64, L), np.float32)
    sinT = np.empty((B, 64, L), np.float32)
    for b in range(B):
        freqs = positions[b].astype(np.float32)[:, None] * inv_freq[None, :]  # [L, 64]
        cosT[b] = np.cos(freqs).T.astype(np.float32)  # [64, L]
        sinT[b] = np.sin(freqs).T.astype(np.float32)

    # single tril tile; diagonal block j's mask is its width-(512-128j) window
    kk = np.arange(128)[:, None]
    qq = np.arange(CHUNK)[None, :]
    mask0 = (kk <= qq).astype(np.float32).astype(ml_dtypes.bfloat16)

    ones_c = np.ones((128, 128), ml_dtypes.bfloat16)

    in_maps = []
    for core in range(NCORES):
        b, g = divmod(core, G)
        hs = slice(g * HG, (g + 1) * HG)
        # W^T for this core's heads: [D, HG*dh]
        wqT = Wq_p[hs].reshape(HG * DH, D).T          # [D, 512]
        wkT = Wk_p[hs].reshape(HG * DH, D).T
        wvT = Wv.reshape(H, DH, D)[hs].reshape(HG * DH, D).T
        # [m][p][kt*128+f] layout
        wq_c = np.ascontiguousarray(
            wqT.reshape(KT, 128, HG, DH).transpose(2, 1, 0, 3).reshape(
                HG, 128, KT * DH)).astype(np.float16)
        wk_c = np.ascontiguousarray(
            wkT.reshape(KT, 128, HG, DH).transpose(2, 1, 0, 3).reshape(
                HG, 128, KT * DH)).astype(np.float16)
        # [kt][p][f]
        wv_c = np.ascontiguousarray(
            wvT.reshape(KT, 128, HG * DH)).astype(np.float16)
        # wo[h][d'][o] = Wo[o, (g*HG+h)*dh + d']
        wo_c = np.ascontiguousarray(
            Wo.T.reshape(H, DH, D)[hs]).astype(ml_dtypes.bfloat16)  # [HG, dh, D]
        xtb = x[b].T.astype(np.float16)   # [D, L]
        xt_tiled = np.ascontiguousarray(
            xtb.reshape(KT, 128, NCH, CHUNK).transpose(2, 1, 0, 3).reshape(
                NCH, 128, KT * CHUNK))
        in_maps.append({
            "xt": xt_tiled,
            "wq": wq_c, "wk": wk_c, "wv": wv_c, "wo": wo_c,
            "cosT": cosT[b].astype(np.float16),
            "sinT": sinT[b].astype(np.float16),
            "masks": mask0,
            "ones_c": ones_c,
        })
    return in_maps


def kernel(x, positions, Wq, Wk, Wv, Wo, _profile=False):
    nc = _get_nc()
    in_maps = _host_prep(x, positions, Wq, Wk, Wv, Wo)
    res = run_bass_kernel_spmd(nc, in_maps, list(range(NCORES)), trace=_profile)
    out = np.zeros((B, L, D), np.float32)
    for core in range(NCORES):
        b = core // G
        out[b] += np.asarray(res.results[core]["out"], np.float32)
    if _profile:
        kernel._last_exec_time_ns = res.exec_time_ns
        kernel._last_trace = res.instructions_and_trace
    return out
